# revision 2
# baseline (speedup 1.0000x reference)
"""Trainium2 Bass kernel for nn_CrossModelAttention (gnn_message_passing).

Distribution (8 NeuronCores, one SPMD NEFF):
  - lm head + LayerNorm: node-sharded (256 nodes/core), AllGather -> full lm^T
  - RGCN x2: relation GEMMs replicated (cheap), edge aggregation sharded by
    dst node (edges sorted by dst on host, per-core index/scale arrays);
    segment-sum done on the PE via per-tile one-hot selection matmuls;
    per-core dst aggregation done as dense PE matmuls against a host-built
    sparse-block adjacency (fp16, mean-normalization folded in).
    AllGather -> full g^T after each layer.
  - Attention: query-sharded (each core does all 8 heads for its 256 query
    nodes); no collective needed for the head merge.
  - Residual + BatchNorm: channel stats partial-summed locally, 1KB
    AllReduce, normalization + classifier local; per-core [256, 8] outputs
    concatenated on host.

Layouts: activations kept feature-major ("T layout", [128 feat partitions,
nodes free]) so per-feature params are per-partition scalars and no
transposes are needed anywhere. Matmul inputs fp16, PSUM/stats fp32.
"""

import os
import sys

if "/opt/trn_rl_repo" not in sys.path:
    sys.path.insert(0, "/opt/trn_rl_repo")

import numpy as np

import concourse.bacc as bacc
import concourse.bass as bass
import concourse.mybir as mybir
import concourse.tile as tile
from concourse.bass_utils import run_bass_kernel_spmd

F32 = mybir.dt.float32
F16 = mybir.dt.float16
I16 = mybir.dt.int16
AF = mybir.ActivationFunctionType
OP = mybir.AluOpType

N = 2048          # nodes (B*S)
D = 1024          # input dim
HID = 128
NR = 3            # relations
NL = 2            # rgcn layers
NH = 8            # heads
DH = 16
NCORES = 8
NPC = N // NCORES  # nodes per core = 256
EPS = 1e-5

LAST_RESULT = None  # BassKernelResults of the most recent run (for test harness)


def _ensure_profile_hook():
    """Install the NTFF profile hook if boot() could not (antenv.axon_hooks
    may be missing from the image). Only matters when BASS_TRACE=1."""
    try:
        try:
            import antenv.axon_hooks as ah
        except ImportError:
            import types
            import antenv
            ah = types.ModuleType("antenv.axon_hooks")
            _box = [None]
            ah.set_axon_ntff_profile_hook = lambda h: _box.__setitem__(0, h)
            ah.get_axon_ntff_profile_hook = lambda: _box[0]
            sys.modules["antenv.axon_hooks"] = ah
            antenv.axon_hooks = ah
        if ah.get_axon_ntff_profile_hook() is None:
            from trn_agent_boot.trn_boot import _ntff_profile_via_ctypes
            hook = _ntff_profile_via_ctypes("/opt/axon/libaxon_pjrt.so")
            if hook is not None:
                ah.set_axon_ntff_profile_hook(hook)
    except Exception:
        pass


def _bcast_ap(dram_ap, parts, free):
    """DMA access pattern broadcasting a [free] dram vector across partitions."""
    return bass.AP(tensor=dram_ap.tensor, offset=dram_ap.offset, ap=[[0, parts], [1, free]])


def _gather_cc_ap(cc):
    """AP over cc_out [R, 128, NPC] reading as [128 feat, R*NPC nodes]."""
    return bass.AP(tensor=cc[:].tensor, offset=0,
                   ap=[[NPC, 128], [128 * NPC, NCORES], [1, NPC]])


def build(nc):
    groups = [list(range(NCORES))]

    # ---------------- dram tensors ----------------
    outT_d = nc.dram_tensor("outT", [D, NPC], F16, kind="ExternalInput")
    lmw_d = nc.dram_tensor("lm_w", [D, HID], F16, kind="ExternalInput")
    lmb_d = nc.dram_tensor("lm_b", [HID], F32, kind="ExternalInput")
    lng_d = nc.dram_tensor("ln_g", [HID], F32, kind="ExternalInput")
    lnb_d = nc.dram_tensor("ln_b", [HID], F32, kind="ExternalInput")
    root_d = nc.dram_tensor("root", [NL, HID, HID], F16, kind="ExternalInput")
    rel_d = nc.dram_tensor("rel", [NL, HID, NR * HID], F16, kind="ExternalInput")
    rgb_d = nc.dram_tensor("rgb", [NL, HID], F32, kind="ExternalInput")
    wq_d = nc.dram_tensor("wq", [HID, HID], F16, kind="ExternalInput")
    wk_d = nc.dram_tensor("wk", [HID, HID], F16, kind="ExternalInput")
    wv_d = nc.dram_tensor("wv", [HID, HID], F16, kind="ExternalInput")
    wop_d = nc.dram_tensor("wop", [DH + 1, NH, HID], F16, kind="ExternalInput")
    boe_d = nc.dram_tensor("boe", [HID], F32, kind="ExternalInput")
    bng_d = nc.dram_tensor("bn_g", [HID], F32, kind="ExternalInput")
    bnb_d = nc.dram_tensor("bn_b", [HID], F32, kind="ExternalInput")
    clsw_d = nc.dram_tensor("cls_w", [HID, NH], F16, kind="ExternalInput")
    clsb_d = nc.dram_tensor("cls_b", [NH], F32, kind="ExternalInput")
    adj_d = nc.dram_tensor("adj", [128, N // 128, NR, NPC], F16, kind="ExternalInput")

    y_d = nc.dram_tensor("y", [NPC, NH], F32, kind="ExternalOutput")

    cc_in = [nc.dram_tensor(f"cci{i}", [128, NPC], F16, kind="Internal") for i in range(3)]
    cc_out = [nc.dram_tensor(f"cco{i}", [NCORES, 128, NPC], F16, kind="Internal",
                             addr_space="Shared") for i in range(3)]
    bn_in = nc.dram_tensor("bni", [128, 2], F32, kind="Internal")
    bn_out = nc.dram_tensor("bno", [128, 2], F32, kind="Internal", addr_space="Shared")

    with tile.TileContext(nc) as tc:
        with tc.tile_pool(name="const", bufs=1) as cst, \
             tc.tile_pool(name="persist", bufs=1) as per, \
             tc.tile_pool(name="work", bufs=2) as wk, \
             tc.tile_pool(name="small", bufs=2) as sm, \
             tc.tile_pool(name="epool", bufs=4) as ep, \
             tc.tile_pool(name="msgp", bufs=1) as mp, \
             tc.tile_pool(name="psA", bufs=2, space="PSUM") as psA, \
             tc.tile_pool(name="psB", bufs=1, space="PSUM") as psB, \
             tc.tile_pool(name="psAcc", bufs=1, space="PSUM") as psAcc, \
             tc.tile_pool(name="psSt", bufs=4, space="PSUM") as psSt:

            # ---------------- constants to SBUF ----------------
            lmw_sb = cst.tile([128, D // 128, HID], F16)
            nc.sync.dma_start(out=lmw_sb[:], in_=lmw_d[:].rearrange("(k p) f -> p k f", p=128))
            outT_sb = cst.tile([128, D // 128, NPC], F16)
            nc.sync.dma_start(out=outT_sb[:], in_=outT_d[:].rearrange("(k p) n -> p k n", p=128))
            root_sb = cst.tile([128, NL, HID], F16)
            nc.sync.dma_start(out=root_sb[:], in_=root_d[:].rearrange("l k f -> k l f"))
            rel_sb = cst.tile([128, NL, NR * HID], F16)
            nc.sync.dma_start(out=rel_sb[:], in_=rel_d[:].rearrange("l k f -> k l f"))
            wq_sb = cst.tile([128, HID], F16)
            nc.sync.dma_start(out=wq_sb[:], in_=wq_d[:])
            wk_sb = cst.tile([128, HID], F16)
            nc.sync.dma_start(out=wk_sb[:], in_=wk_d[:])
            wv_sb = cst.tile([128, HID], F16)
            nc.sync.dma_start(out=wv_sb[:], in_=wv_d[:])

            def vec128(d):
                t = cst.tile([128, 1], F32, tag=f"v_{d.name}")
                nc.sync.dma_start(out=t[:], in_=d[:, None])
                return t
            lmb_sb = vec128(lmb_d)
            lng_sb = vec128(lng_d)
            lnb_sb = vec128(lnb_d)
            bng_sb = vec128(bng_d)
            bnb_sb = vec128(bnb_d)
            boe_sb = vec128(boe_d)
            rgb_sb = cst.tile([128, NL], F32)
            nc.sync.dma_start(out=rgb_sb[:], in_=rgb_d[:].rearrange("l f -> f l"))
            clsb_bc = cst.tile([128, NH], F32)
            nc.gpsimd.dma_start(out=clsb_bc[:], in_=_bcast_ap(clsb_d[:], 128, NH))

            ones_col = cst.tile([128, 1], F32)
            nc.vector.memset(ones_col[:], 1.0)
            ones_1x128 = cst.tile([1, 128], F32)
            nc.vector.memset(ones_1x128[:], 1.0)
            ones_1x17 = cst.tile([1, DH + 1], F32)
            nc.vector.memset(ones_1x17[:], 1.0)
            eps1 = cst.tile([1, 1], F32)
            nc.vector.memset(eps1[:], EPS)
            eps128 = cst.tile([128, 1], F32)
            nc.vector.memset(eps128[:], EPS)

            def ps_acc():
                return psAcc.tile([128, NPC], F32, tag="acc", name="acc")

            def ps_mm():
                return psA.tile([128, NR * HID], F32, tag="mm", name="mm")

            def ps_b():
                return psB.tile([128, 512], F32, tag="b", name="b")

            # ---------------- phase 1: lm head + LN (own nodes) ----------------
            lm_ps = ps_acc()
            for k in range(D // 128):
                nc.tensor.matmul(lm_ps[:], lhsT=lmw_sb[:, k, :], rhs=outT_sb[:, k, :],
                                 start=(k == 0), stop=(k == D // 128 - 1))
            r_sb = per.tile([128, NPC], F32, tag="r")
            nc.scalar.activation(out=r_sb[:], in_=lm_ps[:], func=AF.Relu, bias=lmb_sb[:], scale=1.0)
            sq = wk.tile([128, NPC], F32, tag="sq")
            nc.vector.tensor_mul(sq[:], r_sb[:], r_sb[:])
            st_ps = ps_b()[0:1, :]
            nc.tensor.matmul(st_ps[:, 0:NPC], lhsT=ones_col[:], rhs=r_sb[:], start=True, stop=True)
            nc.tensor.matmul(st_ps[:, NPC:], lhsT=ones_col[:], rhs=sq[:], start=True, stop=True)
            mu_r = sm.tile([1, NPC], F32, tag="mu")
            nc.vector.tensor_scalar_mul(mu_r[:], st_ps[:, 0:NPC], 1.0 / HID)
            ex2_r = sm.tile([1, NPC], F32, tag="ex2")
            nc.vector.tensor_scalar_mul(ex2_r[:], st_ps[:, NPC:], 1.0 / HID)
            var_r = sm.tile([1, NPC], F32, tag="var")
            nc.vector.tensor_mul(var_r[:], mu_r[:], mu_r[:])
            nc.vector.tensor_sub(var_r[:], ex2_r[:], var_r[:])
            sd_r = sm.tile([1, NPC], F32, tag="sd")
            nc.scalar.activation(out=sd_r[:], in_=var_r[:], func=AF.Sqrt, bias=eps1[:], scale=1.0)
            pk = sm.tile([1, 2 * NPC], F32, tag="pk")
            nc.vector.reciprocal(pk[:, 0:NPC], sd_r[:])
            nc.vector.tensor_mul(pk[:, NPC:], mu_r[:], pk[:, 0:NPC])
            bc_ps = ps_b()
            nc.tensor.matmul(bc_ps[:], lhsT=ones_1x128[:], rhs=pk[:], start=True, stop=True)
            lmT_own = per.tile([128, NPC], F32, tag="lmT_own")
            nc.vector.tensor_mul(lmT_own[:], r_sb[:], bc_ps[:, 0:NPC])
            nc.vector.tensor_sub(lmT_own[:], lmT_own[:], bc_ps[:, NPC:])
            nc.vector.tensor_scalar(out=lmT_own[:], in0=lmT_own[:], scalar1=lng_sb[:],
                                    scalar2=lnb_sb[:], op0=OP.mult, op1=OP.add)
            lm16_own = per.tile([128, NPC], F16, tag="lm16_own")
            nc.vector.tensor_copy(lm16_own[:], lmT_own[:])

            adj_sb = cst.tile([128, N // 128, NR, NPC], F16)
            nc.sync.dma_start(out=adj_sb[:], in_=adj_d[:])
            wop_sb = cst.tile([DH + 1, NH, HID], F16)
            nc.sync.dma_start(out=wop_sb[:], in_=wop_d[:])
            cls_sb = cst.tile([128, NH], F16)
            nc.sync.dma_start(out=cls_sb[:], in_=clsw_d[:])

            q16_all = per.tile([DH, NH, NPC], F16, tag="q16a")
            for h in range(NH):
                q_ps = ps_b()[0:DH, 0:NPC]
                nc.tensor.matmul(q_ps[:], lhsT=wq_sb[:, DH * h:DH * (h + 1)], rhs=lm16_own[:],
                                 start=True, stop=True)
                nc.vector.tensor_copy(q16_all[:, h, :], q_ps[:])

            nc.sync.dma_start(out=cc_in[0][:], in_=lm16_own[:])
            nc.gpsimd.collective_compute(
                kind="AllGather", op=OP.bypass, replica_groups=groups,
                ins=[cc_in[0][:]], outs=[cc_out[0][:]])
            lmT_full = per.tile([128, N], F16, tag="lmT_full")
            nc.sync.dma_start(out=lmT_full[:].rearrange("f (r n) -> f r n", r=NCORES), in_=_gather_cc_ap(cc_out[0]))

            # ---------------- phase 2: RGCN layers ----------------
            xT = lmT_full
            x16_own = lm16_own
            g16_own = None
            for l in range(NL):
                xr_all = mp.tile([128, N // 128, NR, HID], F16, tag="xr_all", name="xr_all")
                for c in range(N // 128):
                    xr_ps = ps_mm()
                    nc.tensor.matmul(xr_ps[:], lhsT=xT[:, 128 * c:128 * (c + 1)],
                                     rhs=rel_sb[:, l, :], start=True, stop=True)
                    nc.vector.tensor_copy(
                        xr_all[:, c, :, :],
                        xr_ps[:].rearrange("p (r f) -> p r f", r=NR))
                agg_ps = ps_acc()
                nc.tensor.matmul(agg_ps[:], lhsT=root_sb[:, l, :], rhs=x16_own[:],
                                 start=True, stop=False)
                for c in range(N // 128):
                    for r in range(NR):
                        nc.tensor.matmul(agg_ps[:], lhsT=xr_all[:, c, r, :],
                                         rhs=adj_sb[:, c, r, :], start=False,
                                         stop=(c == N // 128 - 1 and r == NR - 1))
                g16_own = per.tile([128, NPC], F16, tag=f"g16_own{l}")
                nc.scalar.activation(out=g16_own[:], in_=agg_ps[:], func=AF.Relu,
                                     bias=rgb_sb[:, l:l + 1], scale=1.0)
                nc.sync.dma_start(out=cc_in[1 + l][:], in_=g16_own[:])
                nc.gpsimd.collective_compute(
                    kind="AllGather", op=OP.bypass, replica_groups=groups,
                    ins=[cc_in[1 + l][:]], outs=[cc_out[1 + l][:]])
                gT_full = per.tile([128, N], F16, tag=f"gT_full{l}")
                nc.sync.dma_start(out=gT_full[:].rearrange("f (r n) -> f r n", r=NCORES), in_=_gather_cc_ap(cc_out[1 + l]))
                xT = gT_full
                x16_own = g16_own

            gT = xT  # final graph features, feature-major, fp16

            # ---------------- phase 3: attention (all heads, own queries) ----------------
            vaug = per.tile([128, N // 128, NH, DH + 1], F16, tag="vaug")
            nc.vector.memset(vaug[:, :, :, 0:1], 1.0)
            for c in range(N // 128):
                v_ps = ps_mm()[:, 0:HID]
                nc.tensor.matmul(v_ps[:], lhsT=gT[:, 128 * c:128 * (c + 1)], rhs=wv_sb[:],
                                 start=True, stop=True)
                nc.vector.tensor_copy(
                    vaug[:, c, :, 1:DH + 1],
                    v_ps[:].rearrange("p (h d) -> p h d", h=NH))

            attn_ps = ps_acc()
            for h in range(NH):
                q16 = q16_all[:, h, :]
                k16 = wk.tile([DH, N], F16, tag="k16")
                for j in range(N // 512):
                    k_ps = ps_b()[0:DH, :]
                    nc.tensor.matmul(k_ps[:], lhsT=wk_sb[:, DH * h:DH * (h + 1)],
                                     rhs=gT[:, 512 * j:512 * (j + 1)], start=True, stop=True)
                    nc.vector.tensor_copy(k16[:, 512 * j:512 * (j + 1)], k_ps[:])
                num_ps = ps_b()[0:DH + 1, 0:NPC]
                for jp in range(N // 256):
                    st_ps = psSt.tile([128, 2, NPC], F32, tag="stps", name="stps")
                    for u in range(2):
                        j = 2 * jp + u
                        nc.tensor.matmul(st_ps[:, u, :], lhsT=k16[:, 128 * j:128 * (j + 1)],
                                         rhs=q16[:], start=True, stop=True)
                    e16 = ep.tile([128, 2, NPC], F16, tag="e16", name="e16")
                    nc.scalar.activation(out=e16[:], in_=st_ps[:], func=AF.Exp)
                    for u in range(2):
                        j = 2 * jp + u
                        nc.tensor.matmul(num_ps[:], lhsT=vaug[:, j, h, :], rhs=e16[:, u, :],
                                         start=(j == 0), stop=(j == N // 128 - 1))
                num_sb = sm.tile([DH + 1, NPC], F32, tag="num")
                nc.vector.tensor_copy(num_sb[:], num_ps[:])
                rden = sm.tile([1, NPC], F32, tag="rden")
                nc.vector.reciprocal(rden[:], num_sb[0:1, :])
                dbc_ps = ps_b()[0:DH + 1, 0:NPC]
                nc.tensor.matmul(dbc_ps[:], lhsT=ones_1x17[:], rhs=rden[:], start=True, stop=True)
                ctx16 = sm.tile([DH + 1, NPC], F16, tag="ctx16")
                nc.vector.tensor_mul(ctx16[:], num_sb[:], dbc_ps[:])
                nc.tensor.matmul(attn_ps[:], lhsT=wop_sb[:, h, :], rhs=ctx16[:],
                                 start=(h == 0), stop=(h == NH - 1))

            # ---------------- phase 4: residual + BN + classifier ----------------
            fused = per.tile([128, NPC], F32, tag="fused")
            nc.vector.scalar_tensor_tensor(out=fused[:], in0=attn_ps[:], scalar=boe_sb[:],
                                           in1=lmT_own[:], op0=OP.add, op1=OP.add)
            fsq = wk.tile([128, NPC], F32, tag="fsq")
            nc.vector.tensor_mul(fsq[:], fused[:], fused[:])
            bnp = sm.tile([128, 2], F32, tag="bnp")
            nc.vector.tensor_reduce(bnp[:, 0:1], fused[:], mybir.AxisListType.X, OP.add)
            nc.vector.tensor_reduce(bnp[:, 1:2], fsq[:], mybir.AxisListType.X, OP.add)
            nc.sync.dma_start(out=bn_in[:], in_=bnp[:])
            nc.gpsimd.collective_compute(
                kind="AllReduce", op=OP.add, replica_groups=groups,
                ins=[bn_in[:]], outs=[bn_out[:]])
            bnst = sm.tile([128, 2], F32, tag="bnst")
            nc.sync.dma_start(out=bnst[:], in_=bn_out[:])
            mu_c = sm.tile([128, 1], F32, tag="muc")
            nc.vector.tensor_scalar_mul(mu_c[:], bnst[:, 0:1], 1.0 / N)
            var_c = sm.tile([128, 1], F32, tag="varc")
            nc.vector.tensor_scalar_mul(var_c[:], bnst[:, 1:2], 1.0 / N)
            mu2_c = sm.tile([128, 1], F32, tag="mu2c")
            nc.vector.tensor_mul(mu2_c[:], mu_c[:], mu_c[:])
            nc.vector.tensor_sub(var_c[:], var_c[:], mu2_c[:])
            sd_c = sm.tile([128, 1], F32, tag="sdc")
            nc.scalar.activation(out=sd_c[:], in_=var_c[:], func=AF.Sqrt, bias=eps128[:], scale=1.0)
            scl_c = sm.tile([128, 1], F32, tag="sclc")
            nc.vector.reciprocal(scl_c[:], sd_c[:])
            nc.vector.tensor_mul(scl_c[:], scl_c[:], bng_sb[:])
            shf_c = sm.tile([128, 1], F32, tag="shfc")
            nc.vector.tensor_mul(shf_c[:], mu_c[:], scl_c[:])
            nc.vector.tensor_sub(shf_c[:], bnb_sb[:], shf_c[:])
            fn16 = wk.tile([128, NPC], F16, tag="fn16")
            nc.vector.tensor_scalar(out=fn16[:], in0=fused[:], scalar1=scl_c[:],
                                    scalar2=shf_c[:], op0=OP.mult, op1=OP.add)
            yv = y_d[:].rearrange("(c p) f -> c p f", p=128)
            for c in range(NPC // 128):
                lg_ps = ps_mm()[:, 0:NH]
                nc.tensor.matmul(lg_ps[:], lhsT=fn16[:, 128 * c:128 * (c + 1)], rhs=cls_sb[:],
                                 start=True, stop=True)
                out_sb = wk.tile([128, NH], F32, tag="outsb")
                nc.vector.tensor_add(out_sb[:], lg_ps[:], clsb_bc[:])
                nc.sync.dma_start(out=yv[c], in_=out_sb[:])

    nc.finalize()
    return nc


_CACHE = {}


def kernel(output, edge_index, edge_type, lm_w, lm_b, ln_g, ln_b,
           rgcn_root, rgcn_rel, rgcn_bias, wq, bq, wk, bk, wv, bv,
           wo, bo, bn_g, bn_b, cls_w, cls_b):
    global LAST_RESULT
    _ensure_profile_hook()

    output = np.asarray(output, np.float32)
    src = np.asarray(edge_index[0]).astype(np.int64)
    dst = np.asarray(edge_index[1]).astype(np.int64)
    et = np.asarray(edge_type).astype(np.int64)
    bq = np.asarray(bq, np.float32)
    if np.abs(bq).max() > 0:
        raise NotImplementedError("nonzero bq not supported by this kernel")

    # ---- host-side layout prep (index math only) ----
    outT = np.ascontiguousarray(output.reshape(N, D).T).astype(np.float16)  # [D, N]
    cnt = np.zeros((N, NR), np.float32)
    np.add.at(cnt, (dst, et), 1.0)
    scale_e = (1.0 / np.maximum(cnt, 1.0))[dst, et].astype(np.float32)
    # dense sparse-block adjacency per core: adj[p, c, r, d] = sum of
    # 1/max(cnt,1) over edges (src=c*128+p, type=r, dst=core_base+d)
    A = np.zeros((N, NR, N), np.float32)
    np.add.at(A, (src, et, dst), scale_e)
    A = A.reshape(16, 128, NR, NCORES, NPC).transpose(3, 1, 0, 2, 4)  # [core, p, c, r, d]
    per_core = [np.ascontiguousarray(A[c]).astype(np.float16) for c in range(NCORES)]

    wo_pad = np.zeros((DH + 1, NH, HID), np.float32)
    for h in range(NH):
        wo_pad[1:, h, :] = wo[DH * h:DH * (h + 1), :]
    bo_eff = (np.asarray(bo, np.float64) + np.asarray(bv, np.float64) @ np.asarray(wo, np.float64)).astype(np.float32)
    rel_cat = np.concatenate([rgcn_rel[:, r, :, :] for r in range(NR)], axis=2)  # [NL, HID, NR*HID]

    shared = {
        "lm_w": np.asarray(lm_w, np.float16),
        "lm_b": np.asarray(lm_b, np.float32),
        "ln_g": np.asarray(ln_g, np.float32),
        "ln_b": np.asarray(ln_b, np.float32),
        "root": np.asarray(rgcn_root, np.float16),
        "rel": np.ascontiguousarray(rel_cat).astype(np.float16),
        "rgb": np.asarray(rgcn_bias, np.float32),
        "wq": np.asarray(wq, np.float16),
        "wk": np.asarray(wk, np.float16),
        "wv": np.asarray(wv, np.float16),
        "wop": wo_pad.astype(np.float16),
        "boe": bo_eff,
        "bn_g": np.asarray(bn_g, np.float32),
        "bn_b": np.asarray(bn_b, np.float32),
        "cls_w": np.asarray(cls_w, np.float16),
        "cls_b": np.asarray(cls_b, np.float32),
    }

    in_maps = []
    for c in range(NCORES):
        m = dict(shared)
        m["outT"] = np.ascontiguousarray(outT[:, c * NPC:(c + 1) * NPC])
        m["adj"] = per_core[c]
        in_maps.append(m)

    if "nc" not in _CACHE:
        nc = bacc.Bacc("TRN2")
        nc.num_devices = NCORES
        _CACHE["nc"] = build(nc)
    nc = _CACHE["nc"]

    res = run_bass_kernel_spmd(nc, in_maps, core_ids=list(range(NCORES)))
    LAST_RESULT = res
    y = np.concatenate([res.results[c]["y"] for c in range(NCORES)], axis=0)
    return y.reshape(1, N, NH).astype(np.float32)



# revision 23
# speedup vs baseline: 1.0907x; 1.0907x over previous
"""Trainium2 Bass kernel for nn_CrossModelAttention (gnn_message_passing).

Distribution (8 NeuronCores, one SPMD NEFF):
  - lm head + LayerNorm: node-sharded (256 nodes/core), all local.
  - RGCN x2: message passing: each core transforms only its OWN 256 nodes
    (per-relation GEMM), then computes partial messages to ALL 2048 dst via
    fp8 DoubleRow matmuls against a host-built dense adjacency block
    (mean-normalization folded in); ReduceScatter(sum) returns each core its
    own dst slice. Root term overlaps the RS.
  - Attention: query-sharded. Scores for ALL 8 heads per key chunk with one
    matmul against a zero-blocked Q operand (full 128 contraction). exp is
    split across ACT (native, fp8 out) and DVE/GPSIMD (Schraudolph bit-trick
    into fp8). Numerator via fp8 DoubleRow matmuls (2 key chunks per
    instruction), denominators from an augmented ones-row in V.
  - Residual + BatchNorm: 1KB AllReduce of channel partials, local norm +
    classifier.

Layouts: activations feature-major ("T layout"). Heavy matmuls fp8/fp16,
PSUM/stats fp32.
"""

import os
import sys

if "/opt/trn_rl_repo" not in sys.path:
    sys.path.insert(0, "/opt/trn_rl_repo")

import numpy as np

import concourse.bacc as bacc
import concourse.bass as bass
import concourse.mybir as mybir
import concourse.tile as tile
from concourse.bass_utils import run_bass_kernel_spmd

F32 = mybir.dt.float32
F16 = mybir.dt.float16
F8 = mybir.dt.float8e4
I8 = mybir.dt.int8
U8 = mybir.dt.uint8
AF = mybir.ActivationFunctionType
OP = mybir.AluOpType
DR = mybir.MatmulPerfMode.DoubleRow

N = 2048          # nodes (B*S)
D = 1024          # input dim
HID = 128
NR = 3            # relations
NL = 2            # rgcn layers
NH = 8            # heads
DH = 16
DHA = 20       # augmented V width (ones + 16 dims + pad to 4-byte multiple)
NCORES = 8
NPC = N // NCORES  # nodes per core = 256
NCH = N // 128     # key chunks = 16
EPS = 1e-5
LN2 = 0.6931471805599453
LOG2E = 1.4426950408889634
# Schraudolph constant for 2^x in fp8e4m3 bits: bits = x*8 + SCHB
SCHB = 8.0 * (7.0 - 0.0450) + 0.5

# exp engine assignment per (chunk, group) unit index u = c*2+g  (32 units):
# 'act' = native exp on ACT; 'dve' = Schraudolph pass1 on DVE (PSUM->SBUF
# fp16), pass2 int8 cast on GPSIMD (SBUF->SBUF).
EXP_ENG = {}
for _u in range(32):
    EXP_ENG[_u] = 'act' if _u % 2 == 0 else 'dve'

LAST_RESULT = None  # BassKernelResults of the most recent run (for test harness)


def _ensure_profile_hook():
    """Install the NTFF profile hook if boot() could not. Only matters when
    BASS_TRACE=1; degrades silently otherwise."""
    try:
        try:
            import antenv.axon_hooks as ah
        except ImportError:
            import types
            import antenv
            ah = types.ModuleType("antenv.axon_hooks")
            _box = [None]
            ah.set_axon_ntff_profile_hook = lambda h: _box.__setitem__(0, h)
            ah.get_axon_ntff_profile_hook = lambda: _box[0]
            sys.modules["antenv.axon_hooks"] = ah
            antenv.axon_hooks = ah
        if ah.get_axon_ntff_profile_hook() is None:
            from trn_agent_boot.trn_boot import _ntff_profile_via_ctypes
            hook = _ntff_profile_via_ctypes("/opt/axon/libaxon_pjrt.so")
            if hook is not None:
                ah.set_axon_ntff_profile_hook(hook)
    except Exception:
        pass


def _bcast_ap(dram_ap, parts, free):
    return bass.AP(tensor=dram_ap.tensor, offset=dram_ap.offset, ap=[[0, parts], [1, free]])


def _gather_cc_ap(cc):
    """AP over cc_out [R, 128, NPC] reading as [128 feat, R*NPC nodes]."""
    return bass.AP(tensor=cc[:].tensor, offset=0,
                   ap=[[NPC, 128], [128 * NPC, NCORES], [1, NPC]])


def build(nc):
    groups = [list(range(NCORES))]

    # ---------------- dram tensors ----------------
    outT_d = nc.dram_tensor("outT", [D, NPC], F16, kind="ExternalInput")
    lmw_d = nc.dram_tensor("lm_w", [D, HID], F16, kind="ExternalInput")
    lmb_d = nc.dram_tensor("lm_b", [HID], F32, kind="ExternalInput")
    lng_d = nc.dram_tensor("ln_g", [HID], F32, kind="ExternalInput")
    lnb_d = nc.dram_tensor("ln_b", [HID], F32, kind="ExternalInput")
    root_d = nc.dram_tensor("root", [NL, HID, HID], F16, kind="ExternalInput")
    rel_d = nc.dram_tensor("rel", [NL, HID, NR * HID], F16, kind="ExternalInput")
    rgb_d = nc.dram_tensor("rgb", [NL, HID], F32, kind="ExternalInput")
    wq_d = nc.dram_tensor("wq", [HID, HID], F16, kind="ExternalInput")
    wk_d = nc.dram_tensor("wk", [HID, HID], U8, kind="ExternalInput")
    wv_d = nc.dram_tensor("wv", [HID, HID], U8, kind="ExternalInput")
    wo_d = nc.dram_tensor("wo", [DH + 1, NH, HID], F16, kind="ExternalInput")
    boe_d = nc.dram_tensor("boe", [HID], F32, kind="ExternalInput")
    bng_d = nc.dram_tensor("bn_g", [HID], F32, kind="ExternalInput")
    bnb_d = nc.dram_tensor("bn_b", [HID], F32, kind="ExternalInput")
    clsw_d = nc.dram_tensor("cls_w", [HID, NH], F16, kind="ExternalInput")
    clsb_d = nc.dram_tensor("cls_b", [NH], F32, kind="ExternalInput")
    adj_d = nc.dram_tensor("adj", [128, NR, 2, N], U8, kind="ExternalInput")

    y_d = nc.dram_tensor("y", [NPC, NH], F32, kind="ExternalOutput")

    # collectives buffers
    rs_in = [nc.dram_tensor(f"rsi{i}", [NCORES, 128, NPC], F16, kind="Internal")
             for i in range(NL)]
    rs_out = [nc.dram_tensor(f"rso{i}", [128, NPC], F16, kind="Internal")
              for i in range(NL)]
    ag_in = nc.dram_tensor("agi", [128, NPC], U8, kind="Internal")
    ag_out = nc.dram_tensor("ago", [NCORES, 128, NPC], U8, kind="Internal",
                            addr_space="Shared")
    bn_in = nc.dram_tensor("bni", [128, 2], F32, kind="Internal")
    bn_out = nc.dram_tensor("bno", [128, 2], F32, kind="Internal", addr_space="Shared")

    with tile.TileContext(nc) as tc:
        with tc.tile_pool(name="const", bufs=1) as cst, \
             tc.tile_pool(name="persist", bufs=1) as per, \
             tc.tile_pool(name="work", bufs=2) as wk, \
             tc.tile_pool(name="small", bufs=2) as sm:

            # ---------------- constants to SBUF ----------------
            outT_sb = cst.tile([128, D // 128, NPC], F16)
            nc.sync.dma_start(out=outT_sb[:], in_=outT_d[:].rearrange("(k p) n -> p k n", p=128))
            lmw_sb = cst.tile([128, D // 128, HID], F16)
            nc.sync.dma_start(out=lmw_sb[:], in_=lmw_d[:].rearrange("(k p) f -> p k f", p=128))
            adj_sb = cst.tile([128, NR, 2, N], F8)
            nc.sync.dma_start(out=adj_sb[:].bitcast(U8), in_=adj_d[:])

            root_sb = cst.tile([128, NL, HID], F16)
            nc.gpsimd.dma_start(out=root_sb[:], in_=root_d[:].rearrange("l k f -> k l f"))
            rel_sb = cst.tile([128, NL, NR * HID], F16)
            nc.gpsimd.dma_start(out=rel_sb[:], in_=rel_d[:].rearrange("l k f -> k l f"))
            wq_sb = cst.tile([128, HID], F16)
            nc.gpsimd.dma_start(out=wq_sb[:], in_=wq_d[:])
            wk_sb = cst.tile([128, HID], F8)
            nc.gpsimd.dma_start(out=wk_sb[:].bitcast(U8), in_=wk_d[:])
            wv_sb = cst.tile([128, HID], F8)
            nc.gpsimd.dma_start(out=wv_sb[:].bitcast(U8), in_=wv_d[:])
            wo_sb = cst.tile([DH + 1, NH, HID], F16)
            nc.gpsimd.dma_start(out=wo_sb[:], in_=wo_d[:])
            cls_sb = cst.tile([128, NH], F16)
            nc.gpsimd.dma_start(out=cls_sb[:], in_=clsw_d[:])

            def vec128(d):
                t = cst.tile([128, 1], F32, tag=f"v_{d.name}")
                nc.gpsimd.dma_start(out=t[:], in_=d[:, None])
                return t
            lmb_sb = vec128(lmb_d)
            lng_sb = vec128(lng_d)
            lnb_sb = vec128(lnb_d)
            bng_sb = vec128(bng_d)
            bnb_sb = vec128(bnb_d)
            boe_sb = vec128(boe_d)
            rgb_sb = cst.tile([128, NL], F32)
            nc.gpsimd.dma_start(out=rgb_sb[:], in_=rgb_d[:].rearrange("l f -> f l"))
            clsb_bc = cst.tile([128, NH], F32)
            nc.gpsimd.dma_start(out=clsb_bc[:], in_=_bcast_ap(clsb_d[:], 128, NH))

            ones_col = cst.tile([128, 1], F32)
            nc.vector.memset(ones_col[:], 1.0)
            ones_1x128 = cst.tile([1, 128], F16)
            nc.vector.memset(ones_1x128[:], 1.0)
            ones_1x17 = cst.tile([1, DH + 1], F16)
            nc.vector.memset(ones_1x17[:], 1.0)
            eps1 = cst.tile([1, 1], F32)
            nc.vector.memset(eps1[:], EPS)
            eps128 = cst.tile([128, 1], F32)
            nc.vector.memset(eps128[:], EPS)

            # persistent activation tiles
            lmT_own = per.tile([128, NPC], F32, tag="lmT_own")    # LN output fp32
            lm16_own = per.tile([128, NPC], F16, tag="lm16_own")
            qb16 = per.tile([128, NH, NPC], F16, tag="qb16")      # blocked Q
            m16 = per.tile([128, N], F16, tag="m16")              # RGCN message partials
            gT8 = per.tile([128, N], F8, tag="gT8")               # final graph feats (full)
            k16 = per.tile([128, N], F16, tag="k16")              # K all heads [hd, key]
            vaug8 = per.tile([128, NCH, NH, DHA], F8, tag="vaug8")
            e8 = per.tile([128, NCH, NH * NPC], F8, tag="e8")     # [128, 16, 2048]
            num_sb = per.tile([DHA, NH, NPC], F32, tag="num_sb")
            rden = per.tile([1, NH, NPC], F32, tag="rden")
            fused = per.tile([128, NPC], F32, tag="fused")

            # =========== phase 1 + RGCN (psum scope A) ===========
            with tc.tile_pool(name="psPre", bufs=2, space="PSUM") as psP, \
                 tc.tile_pool(name="psRoot", bufs=1, space="PSUM") as psR, \
                 tc.tile_pool(name="psMsg", bufs=2, space="PSUM") as psM:

                def big_ps(name):
                    return psP.tile([128, 512], F32, tag="big", name=name)

                # ---- lm head ----
                lm_ps = big_ps("lm")[:, 0:NPC]
                for k in range(D // 128):
                    nc.tensor.matmul(lm_ps, lhsT=lmw_sb[:, k, :], rhs=outT_sb[:, k, :],
                                     start=(k == 0), stop=(k == D // 128 - 1))
                r_sb = wk.tile([128, NPC], F32, tag="r")
                # relu(x + b) on DVE: (x add b) max 0
                nc.vector.tensor_scalar(out=r_sb[:], in0=lm_ps, scalar1=lmb_sb[:],
                                        scalar2=0.0, op0=OP.add, op1=OP.max)
                # ---- LayerNorm over features (partition dim) via PE stats ----
                sq = wk.tile([128, NPC], F32, tag="sq")
                nc.vector.tensor_mul(sq[:], r_sb[:], r_sb[:])
                st_ps = big_ps("st")[0:1, :]
                nc.tensor.matmul(st_ps[:, 0:NPC], lhsT=ones_col[:], rhs=r_sb[:], start=True, stop=True)
                nc.tensor.matmul(st_ps[:, NPC:], lhsT=ones_col[:], rhs=sq[:], start=True, stop=True)
                mu_r = sm.tile([1, NPC], F32, tag="mu")
                nc.vector.tensor_scalar_mul(mu_r[:], st_ps[:, 0:NPC], 1.0 / HID)
                ex2_r = sm.tile([1, NPC], F32, tag="ex2")
                nc.vector.tensor_scalar_mul(ex2_r[:], st_ps[:, NPC:], 1.0 / HID)
                var_r = sm.tile([1, NPC], F32, tag="var")
                nc.vector.tensor_mul(var_r[:], mu_r[:], mu_r[:])
                nc.vector.tensor_sub(var_r[:], ex2_r[:], var_r[:])
                sd_r = sm.tile([1, NPC], F32, tag="sd")
                nc.scalar.activation(out=sd_r[:], in_=var_r[:], func=AF.Sqrt, bias=eps1[:], scale=1.0)
                pk = sm.tile([1, 2 * NPC], F32, tag="pk")
                nc.vector.reciprocal_approx_fast(out=pk[:, 0:NPC], in_=sd_r[:])
                nc.vector.tensor_mul(pk[:, NPC:], mu_r[:], pk[:, 0:NPC])
                pk16 = sm.tile([1, 2 * NPC], F16, tag="pk16")
                nc.vector.tensor_copy(pk16[:], pk[:])
                bc_ps = big_ps("bc")
                nc.tensor.matmul(bc_ps[:], lhsT=ones_1x128[:], rhs=pk16[:], start=True, stop=True)
                nc.vector.tensor_mul(lmT_own[:], r_sb[:], bc_ps[:, 0:NPC])
                nc.vector.tensor_sub(lmT_own[:], lmT_own[:], bc_ps[:, NPC:])
                nc.vector.tensor_scalar(out=lmT_own[:], in0=lmT_own[:], scalar1=lng_sb[:],
                                        scalar2=lnb_sb[:], op0=OP.mult, op1=OP.add)
                nc.vector.tensor_copy(lm16_own[:], lmT_own[:])

                # ---- Q (all heads, one matmul) + blocked-Q build ----
                q_ps = big_ps("q")[:, 0:NPC]
                nc.tensor.matmul(q_ps, lhsT=wq_sb[:], rhs=lm16_own[:], start=True, stop=True)
                q16 = wk.tile([128, NPC], F16, tag="q16")
                nc.vector.tensor_copy(q16[:], q_ps)
                nc.gpsimd.memset(qb16[:], 0.0)
                for h in range(NH):
                    nc.sync.dma_start(out=qb16[DH * h:DH * (h + 1), h, :],
                                      in_=q16[DH * h:DH * (h + 1), :])

                # ---- RGCN layers (message passing + ReduceScatter) ----
                x16 = lm16_own
                g16 = None
                for l in range(NL):
                    # transform own nodes: xr[src, (r f)] per 128-chunk
                    xr8 = wk.tile([128, 2, NR, HID], F8, tag=f"xr8_{l}")
                    for i in range(2):
                        xr_ps = big_ps(f"xr{l}{i}")[:, 0:NR * HID]
                        nc.tensor.matmul(xr_ps, lhsT=x16[:, 128 * i:128 * (i + 1)],
                                         rhs=rel_sb[:, l, :], start=True, stop=True)
                        nc.vector.tensor_copy(
                            xr8[:, i, :, :],
                            xr_ps.rearrange("p (r f) -> p r f", r=NR))
                    # messages to ALL dst: 4 slabs of 512, 3 DR matmuls each
                    for s in range(4):
                        m_ps = psM.tile([128, 512], F32, tag="m", name=f"m{l}{s}")
                        for r in range(NR):
                            nc.tensor.matmul(
                                m_ps[:], lhsT=xr8[:, :, r, :],
                                rhs=adj_sb[:, r, :, 512 * s:512 * (s + 1)],
                                start=(r == 0), stop=(r == NR - 1), perf_mode=DR)
                        nc.vector.tensor_copy(m16[:, 512 * s:512 * (s + 1)], m_ps[:])
                    # ReduceScatter partials (order: [core, feat, node])
                    nc.sync.dma_start(
                        out=rs_in[l][:].rearrange("c f n -> f c n"),
                        in_=m16[:].rearrange("f (c n) -> f c n", c=NCORES))
                    nc.gpsimd.collective_compute(
                        kind="ReduceScatter", op=OP.add, replica_groups=groups,
                        ins=[rs_in[l][:]], outs=[rs_out[l][:]])
                    # root term overlaps the RS
                    root_ps = psR.tile([128, NPC], F32, tag="root", name=f"root{l}")
                    nc.tensor.matmul(root_ps[:], lhsT=root_sb[:, l, :], rhs=x16[:],
                                     start=True, stop=True)
                    agg_sb = wk.tile([128, NPC], F16, tag=f"agg{l}")
                    nc.sync.dma_start(out=agg_sb[:], in_=rs_out[l][:])
                    pre = wk.tile([128, NPC], F32, tag=f"pre{l}")
                    nc.vector.scalar_tensor_tensor(out=pre[:], in0=root_ps[:],
                                                   scalar=rgb_sb[:, l:l + 1],
                                                   in1=agg_sb[:], op0=OP.add, op1=OP.add)
                    g16 = per.tile([128, NPC], F16, tag=f"g16_{l}")
                    nc.vector.tensor_scalar(out=g16[:], in0=pre[:], scalar1=0.0,
                                            scalar2=0.0, op0=OP.max, op1=OP.add)
                    x16 = g16

                # ---- AllGather final g (fp8 transport) ----
                g8 = wk.tile([128, NPC], F8, tag="g8")
                nc.vector.tensor_copy(g8[:], g16[:])
                nc.sync.dma_start(out=ag_in[:], in_=g8[:].bitcast(U8))
                nc.gpsimd.collective_compute(
                    kind="AllGather", op=OP.bypass, replica_groups=groups,
                    ins=[ag_in[:]], outs=[ag_out[:]])
                nc.sync.dma_start(out=gT8[:].bitcast(U8).rearrange("f (r n) -> f r n", r=NCORES),
                                  in_=_gather_cc_ap(ag_out))

                # ---- K (all heads) and V (key-major, fp8, augmented ones) ----
                for j in range(4):
                    k_ps = big_ps(f"k{j}")
                    nc.tensor.matmul(k_ps[:], lhsT=wk_sb[:], rhs=gT8[:, 512 * j:512 * (j + 1)],
                                     start=True, stop=True)
                    nc.vector.tensor_copy(k16[:, 512 * j:512 * (j + 1)], k_ps[:])
                nc.gpsimd.memset(vaug8[:].bitcast(U8), 0)
                nc.gpsimd.memset(vaug8[:, :, :, 0:1].bitcast(U8), 0x38)  # 1.0 in e4m3
                for c in range(NCH):
                    v_ps = big_ps(f"v{c}")[:, 0:HID]
                    nc.tensor.matmul(v_ps, lhsT=gT8[:, 128 * c:128 * (c + 1)], rhs=wv_sb[:],
                                     start=True, stop=True)
                    nc.vector.tensor_copy(
                        vaug8[:, c, :, 1:DH + 1],
                        v_ps.rearrange("p (h d) -> p h d", h=NH))

            # =========== attention scores/exp/num (psum scope B) ===========
            with tc.tile_pool(name="psS", bufs=2, space="PSUM") as psS, \
                 tc.tile_pool(name="psN", bufs=1, space="PSUM") as psN:

                num_ps = psN.tile([DHA, NH, NPC], F32, tag="num", name="num")
                for c in range(NCH):
                    for g in range(2):
                        s_ps = psS.tile([128, 2, 512], F32, tag="sc", name=f"s{c}{g}")
                        for s2 in range(2):
                            nc.tensor.matmul(
                                s_ps[:, s2, :],
                                lhsT=k16[:, 128 * c:128 * (c + 1)],
                                rhs=qb16[:, 4 * g + 2 * s2:4 * g + 2 * s2 + 2, :],
                                start=True, stop=True)
                        # exp over [128, 1024], engine chosen per unit
                        u = 2 * c + g
                        eng = EXP_ENG[u]
                        e_out = e8[:, c, 1024 * g:1024 * (g + 1)]
                        s_in = s_ps[:].rearrange("p a b -> p (a b)")
                        if eng == 'act':
                            nc.scalar.activation(out=e_out, in_=s_in, func=AF.Exp,
                                                 scale=LN2)
                        else:
                            t16 = wk.tile([128, 1024], F16, tag="t16")
                            nc.vector.tensor_scalar(out=t16[:], in0=s_in, scalar1=8.0,
                                                    scalar2=SCHB, op0=OP.mult, op1=OP.add)
                            nc.gpsimd.tensor_copy(e_out.bitcast(I8), t16[:])
                    # numerator DR matmuls for completed chunk pairs
                    if c % 2 == 1:
                        jp = c // 2
                        for h in range(NH):
                            nc.tensor.matmul(
                                num_ps[:, h, :],
                                lhsT=vaug8[:, 2 * jp:2 * jp + 2, h, :],
                                rhs=e8[:, 2 * jp:2 * jp + 2, NPC * h:NPC * (h + 1)],
                                start=(jp == 0), stop=(jp == NCH // 2 - 1),
                                perf_mode=DR)

                # denominators -> reciprocal (fp32); stash numerators in SBUF
                for h in range(NH):
                    nc.vector.reciprocal_approx_fast(out=rden[:, h, :], in_=num_ps[0:1, h, :])
                nc.vector.tensor_copy(num_sb[:].rearrange("d h n -> d (h n)"),
                                      num_ps[:].rearrange("d h n -> d (h n)"))

            # =========== attention tails + BN + classifier (psum scope C) ===========
            with tc.tile_pool(name="psT", bufs=2, space="PSUM") as psT, \
                 tc.tile_pool(name="psA", bufs=1, space="PSUM") as psA:
                attn_ps = psA.tile([128, NPC], F32, tag="attn", name="attn")
                for h in range(NH):
                    rd16 = sm.tile([1, NPC], F16, tag="rd16")
                    nc.vector.tensor_copy(rd16[:], rden[:, h, :])
                    dbc_ps = psT.tile([DH + 1, NPC], F32, tag="dbc", name=f"dbc{h}")
                    nc.tensor.matmul(dbc_ps[:], lhsT=ones_1x17[:], rhs=rd16[:],
                                     start=True, stop=True)
                    ctx16 = sm.tile([DH + 1, NPC], F16, tag="ctx16")
                    nc.vector.tensor_mul(ctx16[:], num_sb[0:DH + 1, h, :], dbc_ps[:])
                    nc.tensor.matmul(attn_ps[:], lhsT=wo_sb[:, h, :], rhs=ctx16[:],
                                     start=(h == 0), stop=(h == NH - 1))

                # residual + BatchNorm over nodes (channel stats)
                nc.vector.scalar_tensor_tensor(out=fused[:], in0=attn_ps[:], scalar=boe_sb[:],
                                               in1=lmT_own[:], op0=OP.add, op1=OP.add)
                fsq = wk.tile([128, NPC], F32, tag="fsq")
                nc.vector.tensor_mul(fsq[:], fused[:], fused[:])
                bnp = sm.tile([128, 2], F32, tag="bnp")
                nc.vector.tensor_reduce(bnp[:, 0:1], fused[:], mybir.AxisListType.X, OP.add)
                nc.vector.tensor_reduce(bnp[:, 1:2], fsq[:], mybir.AxisListType.X, OP.add)
                nc.sync.dma_start(out=bn_in[:], in_=bnp[:])
                nc.gpsimd.collective_compute(
                    kind="AllReduce", op=OP.add, replica_groups=groups,
                    ins=[bn_in[:]], outs=[bn_out[:]])
                bnst = sm.tile([128, 2], F32, tag="bnst")
                nc.sync.dma_start(out=bnst[:], in_=bn_out[:])
                mu_c = sm.tile([128, 1], F32, tag="muc")
                nc.vector.tensor_scalar_mul(mu_c[:], bnst[:, 0:1], 1.0 / N)
                var_c = sm.tile([128, 1], F32, tag="varc")
                nc.vector.tensor_scalar_mul(var_c[:], bnst[:, 1:2], 1.0 / N)
                mu2_c = sm.tile([128, 1], F32, tag="mu2c")
                nc.vector.tensor_mul(mu2_c[:], mu_c[:], mu_c[:])
                nc.vector.tensor_sub(var_c[:], var_c[:], mu2_c[:])
                sd_c = sm.tile([128, 1], F32, tag="sdc")
                nc.scalar.activation(out=sd_c[:], in_=var_c[:], func=AF.Sqrt, bias=eps128[:], scale=1.0)
                scl_c = sm.tile([128, 1], F32, tag="sclc")
                nc.vector.reciprocal_approx_fast(out=scl_c[:], in_=sd_c[:])
                nc.vector.tensor_mul(scl_c[:], scl_c[:], bng_sb[:])
                shf_c = sm.tile([128, 1], F32, tag="shfc")
                nc.vector.tensor_mul(shf_c[:], mu_c[:], scl_c[:])
                nc.vector.tensor_sub(shf_c[:], bnb_sb[:], shf_c[:])
                fn16 = wk.tile([128, NPC], F16, tag="fn16")
                nc.vector.tensor_scalar(out=fn16[:], in0=fused[:], scalar1=scl_c[:],
                                        scalar2=shf_c[:], op0=OP.mult, op1=OP.add)
                yv = y_d[:].rearrange("(c p) f -> c p f", p=128)
                for c in range(NPC // 128):
                    lg_ps = psT.tile([128, NH], F32, tag="lg", name=f"lg{c}")
                    nc.tensor.matmul(lg_ps[:], lhsT=fn16[:, 128 * c:128 * (c + 1)], rhs=cls_sb[:],
                                     start=True, stop=True)
                    out_sb = wk.tile([128, NH], F32, tag="outsb")
                    nc.vector.tensor_add(out_sb[:], lg_ps[:], clsb_bc[:])
                    nc.sync.dma_start(out=yv[c], in_=out_sb[:])

    nc.finalize()
    return nc


def _to_fp8_bits(x):
    """float array -> fp8 e4m3 bits as uint8."""
    try:
        import ml_dtypes
        return np.asarray(x).astype(ml_dtypes.float8_e4m3fn).view(np.uint8)
    except Exception:
        x = np.asarray(x, np.float32)
        sign = (x < 0).astype(np.uint8) << 7
        ax = np.abs(x)
        ax = np.clip(ax, 0, 448.0)
        e = np.floor(np.log2(np.maximum(ax, 1e-45))).astype(np.int32)
        e = np.clip(e, -9, 8)
        # normal range e >= -6
        en = np.clip(e, -6, 8)
        m = np.round(ax / (2.0 ** en) * 8).astype(np.int32)
        m = np.where(ax == 0, 0, m)
        # m in [8, 16] for normals; handle rounding overflow
        carry = m >= 16
        en = en + carry
        m = np.where(carry, 8, m)
        sub = en <= -7
        bits = np.where(
            sub,
            np.round(ax / 2.0 ** -9).astype(np.int32),
            ((en + 7) << 3) | (m - 8),
        )
        bits = np.clip(bits, 0, 0x7E)
        return (sign | bits.astype(np.uint8)).astype(np.uint8)


_CACHE = {}


def kernel(output, edge_index, edge_type, lm_w, lm_b, ln_g, ln_b,
           rgcn_root, rgcn_rel, rgcn_bias, wq, bq, wk, bk, wv, bv,
           wo, bo, bn_g, bn_b, cls_w, cls_b):
    global LAST_RESULT
    _ensure_profile_hook()

    output = np.asarray(output, np.float32)
    src = np.asarray(edge_index[0]).astype(np.int64)
    dst = np.asarray(edge_index[1]).astype(np.int64)
    et = np.asarray(edge_type).astype(np.int64)
    bq = np.asarray(bq, np.float32)
    bk = np.asarray(bk, np.float32)
    if np.abs(bq).max() > 0 or np.abs(bk).max() > 0:
        raise NotImplementedError("nonzero bq/bk not supported by this kernel")

    # ---- host-side layout prep (index math only) ----
    outT = np.ascontiguousarray(output.reshape(N, D).T).astype(np.float16)  # [D, N]
    cnt = np.zeros((N, NR), np.float32)
    np.add.at(cnt, (dst, et), 1.0)
    scale_e = (1.0 / np.maximum(cnt, 1.0))[dst, et].astype(np.float32)
    # adjacency per core: [core, p, r, i, dst]: src = core*256 + i*128 + p
    A = np.zeros((NCORES, 128, NR, 2, N), np.float32)
    core_i, rem = src // NPC, src % NPC
    np.add.at(A, (core_i, rem % 128, et, rem // 128, dst), scale_e)
    adj_bits = _to_fp8_bits(A)  # [NCORES, 128, NR, 2, N] uint8
    wo_r = np.zeros((DH + 1, NH, HID), np.float32)
    for h in range(NH):
        wo_r[1:, h, :] = wo[DH * h:DH * (h + 1), :]
    bo_eff = (np.asarray(bo, np.float64) + np.asarray(bv, np.float64) @ np.asarray(wo, np.float64)).astype(np.float32)
    rel_cat = np.concatenate([rgcn_rel[:, r, :, :] for r in range(NR)], axis=2)  # [NL, HID, NR*HID]
    wq_s = (np.asarray(wq, np.float32) * LOG2E).astype(np.float16)

    shared = {
        "lm_w": np.asarray(lm_w, np.float16),
        "lm_b": np.asarray(lm_b, np.float32),
        "ln_g": np.asarray(ln_g, np.float32),
        "ln_b": np.asarray(ln_b, np.float32),
        "root": np.asarray(rgcn_root, np.float16),
        "rel": np.ascontiguousarray(rel_cat).astype(np.float16),
        "rgb": np.asarray(rgcn_bias, np.float32),
        "wq": wq_s,
        "wk": _to_fp8_bits(np.asarray(wk, np.float32)),
        "wv": _to_fp8_bits(np.asarray(wv, np.float32)),
        "wo": wo_r.astype(np.float16),
        "boe": bo_eff,
        "bn_g": np.asarray(bn_g, np.float32),
        "bn_b": np.asarray(bn_b, np.float32),
        "cls_w": np.asarray(cls_w, np.float16),
        "cls_b": np.asarray(cls_b, np.float32),
    }

    in_maps = []
    for c in range(NCORES):
        m = dict(shared)
        m["outT"] = np.ascontiguousarray(outT[:, c * NPC:(c + 1) * NPC])
        m["adj"] = np.ascontiguousarray(adj_bits[c])
        in_maps.append(m)

    if "nc" not in _CACHE:
        nc = bacc.Bacc("TRN2")
        nc.num_devices = NCORES
        _CACHE["nc"] = build(nc)
    nc = _CACHE["nc"]

    res = run_bass_kernel_spmd(nc, in_maps, core_ids=list(range(NCORES)))
    LAST_RESULT = res
    y = np.concatenate([res.results[c]["y"] for c in range(NCORES)], axis=0)
    return y.reshape(1, N, NH).astype(np.float32)


# revision 28
# speedup vs baseline: 1.1227x; 1.0293x over previous
"""Trainium2 Bass kernel for nn_CrossModelAttention (gnn_message_passing).

Distribution (8 NeuronCores, one SPMD NEFF):
  - lm head + LayerNorm: node-sharded (256 nodes/core), all local.
  - RGCN x2: message passing: each core transforms only its OWN 256 nodes
    (per-relation GEMM), then computes partial messages to ALL 2048 dst via
    fp8 DoubleRow matmuls against a host-built dense adjacency block
    (mean-normalization folded in); ReduceScatter(sum) returns each core its
    own dst slice. Root term overlaps the RS.
  - Attention: query-sharded. Scores for ALL 8 heads per key chunk with one
    matmul against a zero-blocked Q operand (full 128 contraction). exp is
    split across ACT (native, fp8 out) and DVE/GPSIMD (Schraudolph bit-trick
    into fp8). Numerator via fp8 DoubleRow matmuls (2 key chunks per
    instruction), denominators from an augmented ones-row in V.
  - Residual + BatchNorm: 1KB AllReduce of channel partials, local norm +
    classifier.

Layouts: activations feature-major ("T layout"). Heavy matmuls fp8/fp16,
PSUM/stats fp32.
"""

import os
import sys

if "/opt/trn_rl_repo" not in sys.path:
    sys.path.insert(0, "/opt/trn_rl_repo")

import numpy as np

import concourse.bacc as bacc
import concourse.bass as bass
import concourse.mybir as mybir
import concourse.tile as tile
from concourse.bass_utils import run_bass_kernel_spmd

F32 = mybir.dt.float32
F16 = mybir.dt.float16
F8 = mybir.dt.float8e4
I8 = mybir.dt.int8
U8 = mybir.dt.uint8
AF = mybir.ActivationFunctionType
OP = mybir.AluOpType
DR = mybir.MatmulPerfMode.DoubleRow

N = 2048          # nodes (B*S)
D = 1024          # input dim
HID = 128
NR = 3            # relations
NL = 2            # rgcn layers
NH = 8            # heads
DH = 16
DHA = 20       # augmented V width (ones + 16 dims + pad to 4-byte multiple)
NCORES = 8
NPC = N // NCORES  # nodes per core = 256
NCH = N // 128     # key chunks = 16
EPS = 1e-5
LN2 = 0.6931471805599453
LOG2E = 1.4426950408889634
# Schraudolph constant for 2^x in fp8e4m3 bits: bits = x*8 + SCHB
SCHB = 8.0 * (7.0 - 0.0450) + 0.5

# exp engine assignment per (chunk, group) unit index u = c*2+g  (32 units):
# 'act' = native exp on ACT (fp8 out); 'dve' = Schraudolph 2-pass on DVE.
EXP_ENG = {}
for _u in range(32):
    EXP_ENG[_u] = 'act' if _u % 8 < 5 else 'dve'

LAST_RESULT = None  # BassKernelResults of the most recent run (for test harness)


def _ensure_profile_hook():
    """Install the NTFF profile hook if boot() could not. Only matters when
    BASS_TRACE=1; degrades silently otherwise."""
    try:
        try:
            import antenv.axon_hooks as ah
        except ImportError:
            import types
            import antenv
            ah = types.ModuleType("antenv.axon_hooks")
            _box = [None]
            ah.set_axon_ntff_profile_hook = lambda h: _box.__setitem__(0, h)
            ah.get_axon_ntff_profile_hook = lambda: _box[0]
            sys.modules["antenv.axon_hooks"] = ah
            antenv.axon_hooks = ah
        if ah.get_axon_ntff_profile_hook() is None:
            from trn_agent_boot.trn_boot import _ntff_profile_via_ctypes
            hook = _ntff_profile_via_ctypes("/opt/axon/libaxon_pjrt.so")
            if hook is not None:
                ah.set_axon_ntff_profile_hook(hook)
    except Exception:
        pass


def _bcast_ap(dram_ap, parts, free):
    return bass.AP(tensor=dram_ap.tensor, offset=dram_ap.offset, ap=[[0, parts], [1, free]])


def _gather_cc_ap(cc):
    """AP over cc_out [R, 128, NPC] reading as [128 feat, R*NPC nodes]."""
    return bass.AP(tensor=cc[:].tensor, offset=0,
                   ap=[[NPC, 128], [128 * NPC, NCORES], [1, NPC]])


def build(nc):
    groups = [list(range(NCORES))]

    # ---------------- dram tensors ----------------
    outT_d = nc.dram_tensor("outT", [D, NPC], F16, kind="ExternalInput")
    lmw_d = nc.dram_tensor("lm_w", [D, HID], F16, kind="ExternalInput")
    lmb_d = nc.dram_tensor("lm_b", [HID], F32, kind="ExternalInput")
    lng_d = nc.dram_tensor("ln_g", [HID], F32, kind="ExternalInput")
    lnb_d = nc.dram_tensor("ln_b", [HID], F32, kind="ExternalInput")
    root_d = nc.dram_tensor("root", [NL, HID, HID], F16, kind="ExternalInput")
    rel_d = nc.dram_tensor("rel", [NL, HID, NR * HID], F16, kind="ExternalInput")
    rgb_d = nc.dram_tensor("rgb", [NL, HID], F32, kind="ExternalInput")
    wq_d = nc.dram_tensor("wq", [HID, HID], F16, kind="ExternalInput")
    wk_d = nc.dram_tensor("wk", [HID, HID], U8, kind="ExternalInput")
    wv_d = nc.dram_tensor("wv", [HID, HID], U8, kind="ExternalInput")
    wo_d = nc.dram_tensor("wo", [DH + 1, NH, HID], F16, kind="ExternalInput")
    boe_d = nc.dram_tensor("boe", [HID], F32, kind="ExternalInput")
    bng_d = nc.dram_tensor("bn_g", [HID], F32, kind="ExternalInput")
    bnb_d = nc.dram_tensor("bn_b", [HID], F32, kind="ExternalInput")
    clsw_d = nc.dram_tensor("cls_w", [HID, NH], F16, kind="ExternalInput")
    clsb_d = nc.dram_tensor("cls_b", [NH], F32, kind="ExternalInput")
    adj_d = nc.dram_tensor("adj", [128, NCH, NR, NPC], U8, kind="ExternalInput")
    eye_d = nc.dram_tensor("eye", [128, 128], F16, kind="ExternalInput")

    y_d = nc.dram_tensor("y", [NPC, NH], F32, kind="ExternalOutput")

    # collectives buffers (3 fp8 AllGathers: lm node-major, g1 node-major,
    # g2 feature-major), BN AllReduce, and a warm-up AllReduce.
    agx_in = [nc.dram_tensor(f"agxi{i}", [128, NPC], U8, kind="Internal")
              for i in range(3)]
    agx_out = [nc.dram_tensor(f"agxo{i}", [NCORES, 128, NPC], U8, kind="Internal",
                              addr_space="Shared") for i in range(3)]
    bn_in = nc.dram_tensor("bni", [128, 2], F32, kind="Internal")
    bn_out = nc.dram_tensor("bno", [128, 2], F32, kind="Internal", addr_space="Shared")
    wu_in = nc.dram_tensor("wui", [1, 4], F32, kind="Internal")
    wu_out = nc.dram_tensor("wuo", [1, 4], F32, kind="Internal", addr_space="Shared")

    with tile.TileContext(nc) as tc:
        with tc.tile_pool(name="const", bufs=1) as cst, \
             tc.tile_pool(name="persist", bufs=1) as per, \
             tc.tile_pool(name="work", bufs=2) as wk, \
             tc.tile_pool(name="small", bufs=2) as sm:

            # ---------------- constants to SBUF ----------------
            outT_sb = cst.tile([128, D // 128, NPC], F16)
            nc.sync.dma_start(out=outT_sb[:], in_=outT_d[:].rearrange("(k p) n -> p k n", p=128))
            lmw_sb = cst.tile([128, D // 128, HID], F16)
            nc.sync.dma_start(out=lmw_sb[:], in_=lmw_d[:].rearrange("(k p) f -> p k f", p=128))
            adj_sb = cst.tile([128, NCH, NR, NPC], F8)
            nc.sync.dma_start(out=adj_sb[:].bitcast(U8), in_=adj_d[:])
            eye_sb = cst.tile([128, 128], F16)
            nc.sync.dma_start(out=eye_sb[:], in_=eye_d[:])
            # warm up the collectives stack immediately (absorbs the first-
            # collective barrier behind the input DMAs + phase 1 compute)
            nc.gpsimd.collective_compute(
                kind="AllReduce", op=OP.add, replica_groups=groups,
                ins=[wu_in[:]], outs=[wu_out[:]])

            root_sb = cst.tile([128, NL, HID], F16)
            nc.gpsimd.dma_start(out=root_sb[:], in_=root_d[:].rearrange("l k f -> k l f"))
            rel_sb = cst.tile([128, NL, NR * HID], F16)
            nc.gpsimd.dma_start(out=rel_sb[:], in_=rel_d[:].rearrange("l k f -> k l f"))
            wq_sb = cst.tile([128, HID], F16)
            nc.gpsimd.dma_start(out=wq_sb[:], in_=wq_d[:])
            wk_sb = cst.tile([128, HID], F8)
            nc.gpsimd.dma_start(out=wk_sb[:].bitcast(U8), in_=wk_d[:])
            wv_sb = cst.tile([128, HID], F8)
            nc.gpsimd.dma_start(out=wv_sb[:].bitcast(U8), in_=wv_d[:])
            wo_sb = cst.tile([DH + 1, NH, HID], F16)
            nc.gpsimd.dma_start(out=wo_sb[:], in_=wo_d[:])
            cls_sb = cst.tile([128, NH], F16)
            nc.gpsimd.dma_start(out=cls_sb[:], in_=clsw_d[:])

            def vec128(d):
                t = cst.tile([128, 1], F32, tag=f"v_{d.name}")
                nc.gpsimd.dma_start(out=t[:], in_=d[:, None])
                return t
            lmb_sb = vec128(lmb_d)
            lng_sb = vec128(lng_d)
            lnb_sb = vec128(lnb_d)
            bng_sb = vec128(bng_d)
            bnb_sb = vec128(bnb_d)
            boe_sb = vec128(boe_d)
            rgb_sb = cst.tile([128, NL], F32)
            nc.gpsimd.dma_start(out=rgb_sb[:], in_=rgb_d[:].rearrange("l f -> f l"))
            clsb_bc = cst.tile([128, NH], F32)
            nc.gpsimd.dma_start(out=clsb_bc[:], in_=_bcast_ap(clsb_d[:], 128, NH))

            ones_col = cst.tile([128, 1], F32)
            nc.vector.memset(ones_col[:], 1.0)
            ones_1x128 = cst.tile([1, 128], F16)
            nc.vector.memset(ones_1x128[:], 1.0)
            ones_1x17 = cst.tile([1, DH + 1], F16)
            nc.vector.memset(ones_1x17[:], 1.0)
            eps1 = cst.tile([1, 1], F32)
            nc.vector.memset(eps1[:], EPS)
            eps128 = cst.tile([128, 1], F32)
            nc.vector.memset(eps128[:], EPS)

            # persistent activation tiles
            lmT_own = per.tile([128, NPC], F32, tag="lmT_own")    # LN output fp32
            lm16_own = per.tile([128, NPC], F16, tag="lm16_own")
            qb16 = per.tile([128, NH, NPC], F16, tag="qb16")      # blocked Q
            x8nm = [per.tile([128, NCH, 128], F8, tag=f"x8nm{l}", name=f"x8nm{l}")
                    for l in range(NL)]   # node-major x (full graph, fp8)
            gT8 = per.tile([128, N], F8, tag="gT8")               # final graph feats (full)
            k16 = per.tile([128, N], F16, tag="k16")              # K all heads [hd, key]
            vaug8 = per.tile([128, NCH, NH, DHA], F8, tag="vaug8")
            e8 = per.tile([128, NCH, NH * NPC], F8, tag="e8")     # [128, 16, 2048]
            num_sb = per.tile([DHA, NH, NPC], F32, tag="num_sb")
            rden = per.tile([1, NH, NPC], F32, tag="rden")
            fused = per.tile([128, NPC], F32, tag="fused")

            # =========== phase 1 + RGCN (psum scope A) ===========
            with tc.tile_pool(name="psPre", bufs=2, space="PSUM") as psP, \
                 tc.tile_pool(name="psAgg", bufs=1, space="PSUM") as psAgg, \
                 tc.tile_pool(name="psG", bufs=2, space="PSUM") as psG:

                def big_ps(name):
                    return psP.tile([128, 512], F32, tag="big", name=name)

                # ---- lm head ----
                lm_ps = big_ps("lm")[:, 0:NPC]
                for k in range(D // 128):
                    nc.tensor.matmul(lm_ps, lhsT=lmw_sb[:, k, :], rhs=outT_sb[:, k, :],
                                     start=(k == 0), stop=(k == D // 128 - 1))
                r_sb = wk.tile([128, NPC], F32, tag="r")
                # relu(x + b) on DVE: (x add b) max 0
                nc.vector.tensor_scalar(out=r_sb[:], in0=lm_ps, scalar1=lmb_sb[:],
                                        scalar2=0.0, op0=OP.add, op1=OP.max)
                # ---- LayerNorm over features (partition dim) via PE stats ----
                sq = wk.tile([128, NPC], F32, tag="sq")
                nc.vector.tensor_mul(sq[:], r_sb[:], r_sb[:])
                st_ps = big_ps("st")[0:1, :]
                nc.tensor.matmul(st_ps[:, 0:NPC], lhsT=ones_col[:], rhs=r_sb[:], start=True, stop=True)
                nc.tensor.matmul(st_ps[:, NPC:], lhsT=ones_col[:], rhs=sq[:], start=True, stop=True)
                mu_r = sm.tile([1, NPC], F32, tag="mu")
                nc.vector.tensor_scalar_mul(mu_r[:], st_ps[:, 0:NPC], 1.0 / HID)
                ex2_r = sm.tile([1, NPC], F32, tag="ex2")
                nc.vector.tensor_scalar_mul(ex2_r[:], st_ps[:, NPC:], 1.0 / HID)
                var_r = sm.tile([1, NPC], F32, tag="var")
                nc.vector.tensor_mul(var_r[:], mu_r[:], mu_r[:])
                nc.vector.tensor_sub(var_r[:], ex2_r[:], var_r[:])
                sd_r = sm.tile([1, NPC], F32, tag="sd")
                nc.scalar.activation(out=sd_r[:], in_=var_r[:], func=AF.Sqrt, bias=eps1[:], scale=1.0)
                pk = sm.tile([1, 2 * NPC], F32, tag="pk")
                nc.vector.reciprocal_approx_fast(out=pk[:, 0:NPC], in_=sd_r[:])
                nc.vector.tensor_mul(pk[:, NPC:], mu_r[:], pk[:, 0:NPC])
                pk16 = sm.tile([1, 2 * NPC], F16, tag="pk16")
                nc.vector.tensor_copy(pk16[:], pk[:])
                bc_ps = big_ps("bc")
                nc.tensor.matmul(bc_ps[:], lhsT=ones_1x128[:], rhs=pk16[:], start=True, stop=True)
                nc.vector.tensor_mul(lmT_own[:], r_sb[:], bc_ps[:, 0:NPC])
                nc.vector.tensor_sub(lmT_own[:], lmT_own[:], bc_ps[:, NPC:])
                nc.vector.tensor_scalar(out=lmT_own[:], in0=lmT_own[:], scalar1=lng_sb[:],
                                        scalar2=lnb_sb[:], op0=OP.mult, op1=OP.add)
                nc.vector.tensor_copy(lm16_own[:], lmT_own[:])

                # ---- Q (all heads, one matmul) + blocked-Q build ----
                q_ps = big_ps("q")[:, 0:NPC]
                nc.tensor.matmul(q_ps, lhsT=wq_sb[:], rhs=lm16_own[:], start=True, stop=True)
                q16 = wk.tile([128, NPC], F16, tag="q16")
                nc.vector.tensor_copy(q16[:], q_ps)
                nc.gpsimd.memset(qb16[:], 0.0)
                for h in range(NH):
                    nc.sync.dma_start(out=qb16[DH * h:DH * (h + 1), h, :],
                                      in_=q16[DH * h:DH * (h + 1), :])

                # ---- RGCN layers (aggregate-first, fp8 node-major AllGathers) ----
                def nm_allgather(x16_src, slot, dst_tile):
                    # transpose own 2 chunks to node-major fp8, AllGather to full
                    xo = wk.tile([128, 2, 128], F8, tag="xnm_own")
                    for i in range(2):
                        tr_ps = psP.tile([128, 128], F16, tag="tr", name=f"tr{slot}{i}")
                        nc.tensor.transpose(tr_ps[:], x16_src[:, 128 * i:128 * (i + 1)],
                                            eye_sb[:])
                        nc.vector.tensor_copy(xo[:, i, :], tr_ps[:])
                    nc.sync.dma_start(out=agx_in[slot][:],
                                      in_=xo[:].rearrange("p i f -> p (i f)").bitcast(U8))
                    nc.gpsimd.collective_compute(
                        kind="AllGather", op=OP.bypass, replica_groups=groups,
                        ins=[agx_in[slot][:]], outs=[agx_out[slot][:]])
                    nc.sync.dma_start(
                        out=dst_tile[:].rearrange("p c f -> p (c f)").bitcast(U8)
                            .rearrange("p (r n) -> p r n", r=NCORES),
                        in_=_gather_cc_ap(agx_out[slot]))

                nm_allgather(lm16_own, 0, x8nm[0])

                x16 = lm16_own
                g16 = None
                for l in range(NL):
                    # aggregate: agg_r[f, dst_own] = sum_src x[src, f] * A_r[src, dst]
                    agg_ps = psAgg.tile([128, NR, NPC], F32, tag="agg", name=f"agg{l}")
                    for u in range(NCH // 2):
                        for r in range(NR):
                            nc.tensor.matmul(
                                agg_ps[:, r, :],
                                lhsT=x8nm[l][:, 2 * u:2 * u + 2, :],
                                rhs=adj_sb[:, 2 * u:2 * u + 2, r, :],
                                start=(u == 0), stop=(u == NCH // 2 - 1),
                                perf_mode=DR)
                    agg16 = wk.tile([128, NR, NPC], F16, tag=f"agg16_{l}")
                    nc.vector.tensor_copy(
                        agg16[:].rearrange("p r n -> p (r n)"),
                        agg_ps[:].rearrange("p r n -> p (r n)"))
                    # g = relu(root^T x + b + sum_r W_r^T agg_r)
                    g_ps = psG.tile([128, NPC], F32, tag="g", name=f"g{l}")
                    nc.tensor.matmul(g_ps[:], lhsT=root_sb[:, l, :], rhs=x16[:],
                                     start=True, stop=False)
                    for r in range(NR):
                        nc.tensor.matmul(g_ps[:], lhsT=rel_sb[:, l, 128 * r:128 * (r + 1)],
                                         rhs=agg16[:, r, :],
                                         start=False, stop=(r == NR - 1))
                    g16 = per.tile([128, NPC], F16, tag=f"g16_{l}")
                    nc.vector.tensor_scalar(out=g16[:], in0=g_ps[:], scalar1=rgb_sb[:, l:l + 1],
                                            scalar2=0.0, op0=OP.add, op1=OP.max)
                    if l < NL - 1:
                        nm_allgather(g16, 1, x8nm[l + 1])
                    x16 = g16

                # ---- AllGather final g (fp8, feature-major) ----
                g8 = wk.tile([128, NPC], F8, tag="g8")
                nc.vector.tensor_copy(g8[:], g16[:])
                nc.sync.dma_start(out=agx_in[2][:], in_=g8[:].bitcast(U8))
                nc.gpsimd.collective_compute(
                    kind="AllGather", op=OP.bypass, replica_groups=groups,
                    ins=[agx_in[2][:]], outs=[agx_out[2][:]])
                nc.sync.dma_start(out=gT8[:].bitcast(U8).rearrange("f (r n) -> f r n", r=NCORES),
                                  in_=_gather_cc_ap(agx_out[2]))

                # ---- K (all heads) and V (key-major, fp8, augmented ones) ----
                for j in range(4):
                    k_ps = big_ps(f"k{j}")
                    nc.tensor.matmul(k_ps[:], lhsT=wk_sb[:], rhs=gT8[:, 512 * j:512 * (j + 1)],
                                     start=True, stop=True)
                    nc.vector.tensor_copy(k16[:, 512 * j:512 * (j + 1)], k_ps[:])
                nc.gpsimd.memset(vaug8[:].bitcast(U8), 0)
                nc.gpsimd.memset(vaug8[:, :, :, 0:1].bitcast(U8), 0x38)  # 1.0 in e4m3
                for c in range(NCH):
                    v_ps = big_ps(f"v{c}")[:, 0:HID]
                    nc.tensor.matmul(v_ps, lhsT=gT8[:, 128 * c:128 * (c + 1)], rhs=wv_sb[:],
                                     start=True, stop=True)
                    nc.vector.tensor_copy(
                        vaug8[:, c, :, 1:DH + 1],
                        v_ps.rearrange("p (h d) -> p h d", h=NH))

            # =========== attention scores/exp/num (psum scope B) ===========
            with tc.tile_pool(name="psS", bufs=2, space="PSUM") as psS, \
                 tc.tile_pool(name="psN", bufs=1, space="PSUM") as psN:

                num_ps = psN.tile([DHA, NH, NPC], F32, tag="num", name="num")
                for c in range(NCH):
                    for g in range(2):
                        s_ps = psS.tile([128, 2, 512], F32, tag="sc", name=f"s{c}{g}")
                        for s2 in range(2):
                            nc.tensor.matmul(
                                s_ps[:, s2, :],
                                lhsT=k16[:, 128 * c:128 * (c + 1)],
                                rhs=qb16[:, 4 * g + 2 * s2:4 * g + 2 * s2 + 2, :],
                                start=True, stop=True)
                        # exp over [128, 1024], engine chosen per unit
                        u = 2 * c + g
                        eng = EXP_ENG[u]
                        e_out = e8[:, c, 1024 * g:1024 * (g + 1)]
                        s_in = s_ps[:].rearrange("p a b -> p (a b)")
                        if eng == 'act':
                            nc.scalar.activation(out=e_out, in_=s_in, func=AF.Exp,
                                                 scale=LN2)
                        else:
                            t16 = wk.tile([128, 1024], F16, tag="t16")
                            nc.vector.tensor_scalar(out=t16[:], in0=s_in, scalar1=8.0,
                                                    scalar2=SCHB, op0=OP.mult, op1=OP.add)
                            nc.vector.tensor_copy(e_out.bitcast(I8), t16[:])
                    # numerator DR matmuls for completed chunk pairs
                    if c % 2 == 1:
                        jp = c // 2
                        for h in range(NH):
                            nc.tensor.matmul(
                                num_ps[:, h, :],
                                lhsT=vaug8[:, 2 * jp:2 * jp + 2, h, :],
                                rhs=e8[:, 2 * jp:2 * jp + 2, NPC * h:NPC * (h + 1)],
                                start=(jp == 0), stop=(jp == NCH // 2 - 1),
                                perf_mode=DR)

                # denominators -> reciprocal (fp32); stash numerators in SBUF
                for h in range(NH):
                    nc.vector.reciprocal_approx_fast(out=rden[:, h, :], in_=num_ps[0:1, h, :])
                nc.vector.tensor_copy(num_sb[:].rearrange("d h n -> d (h n)"),
                                      num_ps[:].rearrange("d h n -> d (h n)"))

            # =========== attention tails + BN + classifier (psum scope C) ===========
            with tc.tile_pool(name="psT", bufs=2, space="PSUM") as psT, \
                 tc.tile_pool(name="psA", bufs=1, space="PSUM") as psA:
                attn_ps = psA.tile([128, NPC], F32, tag="attn", name="attn")
                for h in range(NH):
                    rd16 = sm.tile([1, NPC], F16, tag="rd16")
                    nc.vector.tensor_copy(rd16[:], rden[:, h, :])
                    dbc_ps = psT.tile([DH + 1, NPC], F32, tag="dbc", name=f"dbc{h}")
                    nc.tensor.matmul(dbc_ps[:], lhsT=ones_1x17[:], rhs=rd16[:],
                                     start=True, stop=True)
                    ctx16 = sm.tile([DH + 1, NPC], F16, tag="ctx16")
                    nc.vector.tensor_mul(ctx16[:], num_sb[0:DH + 1, h, :], dbc_ps[:])
                    nc.tensor.matmul(attn_ps[:], lhsT=wo_sb[:, h, :], rhs=ctx16[:],
                                     start=(h == 0), stop=(h == NH - 1))

                # residual + BatchNorm over nodes (channel stats)
                nc.vector.scalar_tensor_tensor(out=fused[:], in0=attn_ps[:], scalar=boe_sb[:],
                                               in1=lmT_own[:], op0=OP.add, op1=OP.add)
                fsq = wk.tile([128, NPC], F32, tag="fsq")
                nc.vector.tensor_mul(fsq[:], fused[:], fused[:])
                bnp = sm.tile([128, 2], F32, tag="bnp")
                nc.vector.tensor_reduce(bnp[:, 0:1], fused[:], mybir.AxisListType.X, OP.add)
                nc.vector.tensor_reduce(bnp[:, 1:2], fsq[:], mybir.AxisListType.X, OP.add)
                nc.sync.dma_start(out=bn_in[:], in_=bnp[:])
                nc.gpsimd.collective_compute(
                    kind="AllReduce", op=OP.add, replica_groups=groups,
                    ins=[bn_in[:]], outs=[bn_out[:]])
                bnst = sm.tile([128, 2], F32, tag="bnst")
                nc.sync.dma_start(out=bnst[:], in_=bn_out[:])
                mu_c = sm.tile([128, 1], F32, tag="muc")
                nc.vector.tensor_scalar_mul(mu_c[:], bnst[:, 0:1], 1.0 / N)
                var_c = sm.tile([128, 1], F32, tag="varc")
                nc.vector.tensor_scalar_mul(var_c[:], bnst[:, 1:2], 1.0 / N)
                mu2_c = sm.tile([128, 1], F32, tag="mu2c")
                nc.vector.tensor_mul(mu2_c[:], mu_c[:], mu_c[:])
                nc.vector.tensor_sub(var_c[:], var_c[:], mu2_c[:])
                sd_c = sm.tile([128, 1], F32, tag="sdc")
                nc.scalar.activation(out=sd_c[:], in_=var_c[:], func=AF.Sqrt, bias=eps128[:], scale=1.0)
                scl_c = sm.tile([128, 1], F32, tag="sclc")
                nc.vector.reciprocal_approx_fast(out=scl_c[:], in_=sd_c[:])
                nc.vector.tensor_mul(scl_c[:], scl_c[:], bng_sb[:])
                shf_c = sm.tile([128, 1], F32, tag="shfc")
                nc.vector.tensor_mul(shf_c[:], mu_c[:], scl_c[:])
                nc.vector.tensor_sub(shf_c[:], bnb_sb[:], shf_c[:])
                fn16 = wk.tile([128, NPC], F16, tag="fn16")
                nc.vector.tensor_scalar(out=fn16[:], in0=fused[:], scalar1=scl_c[:],
                                        scalar2=shf_c[:], op0=OP.mult, op1=OP.add)
                yv = y_d[:].rearrange("(c p) f -> c p f", p=128)
                for c in range(NPC // 128):
                    lg_ps = psT.tile([128, NH], F32, tag="lg", name=f"lg{c}")
                    nc.tensor.matmul(lg_ps[:], lhsT=fn16[:, 128 * c:128 * (c + 1)], rhs=cls_sb[:],
                                     start=True, stop=True)
                    out_sb = wk.tile([128, NH], F32, tag="outsb")
                    nc.vector.tensor_add(out_sb[:], lg_ps[:], clsb_bc[:])
                    nc.sync.dma_start(out=yv[c], in_=out_sb[:])

    nc.finalize()
    return nc


def _to_fp8_bits(x):
    """float array -> fp8 e4m3 bits as uint8."""
    try:
        import ml_dtypes
        return np.asarray(x).astype(ml_dtypes.float8_e4m3fn).view(np.uint8)
    except Exception:
        x = np.asarray(x, np.float32)
        sign = (x < 0).astype(np.uint8) << 7
        ax = np.abs(x)
        ax = np.clip(ax, 0, 448.0)
        e = np.floor(np.log2(np.maximum(ax, 1e-45))).astype(np.int32)
        e = np.clip(e, -9, 8)
        # normal range e >= -6
        en = np.clip(e, -6, 8)
        m = np.round(ax / (2.0 ** en) * 8).astype(np.int32)
        m = np.where(ax == 0, 0, m)
        # m in [8, 16] for normals; handle rounding overflow
        carry = m >= 16
        en = en + carry
        m = np.where(carry, 8, m)
        sub = en <= -7
        bits = np.where(
            sub,
            np.round(ax / 2.0 ** -9).astype(np.int32),
            ((en + 7) << 3) | (m - 8),
        )
        bits = np.clip(bits, 0, 0x7E)
        return (sign | bits.astype(np.uint8)).astype(np.uint8)


_CACHE = {}


def kernel(output, edge_index, edge_type, lm_w, lm_b, ln_g, ln_b,
           rgcn_root, rgcn_rel, rgcn_bias, wq, bq, wk, bk, wv, bv,
           wo, bo, bn_g, bn_b, cls_w, cls_b):
    global LAST_RESULT
    _ensure_profile_hook()

    output = np.asarray(output, np.float32)
    src = np.asarray(edge_index[0]).astype(np.int64)
    dst = np.asarray(edge_index[1]).astype(np.int64)
    et = np.asarray(edge_type).astype(np.int64)
    bq = np.asarray(bq, np.float32)
    bk = np.asarray(bk, np.float32)
    if np.abs(bq).max() > 0 or np.abs(bk).max() > 0:
        raise NotImplementedError("nonzero bq/bk not supported by this kernel")

    # ---- host-side layout prep (index math only) ----
    outT = np.ascontiguousarray(output.reshape(N, D).T).astype(np.float16)  # [D, N]
    cnt = np.zeros((N, NR), np.float32)
    np.add.at(cnt, (dst, et), 1.0)
    scale_e = (1.0 / np.maximum(cnt, 1.0))[dst, et].astype(np.float32)
    # adjacency per core: [core, p, r, i, dst]: src = core*256 + i*128 + p
    # adj[core, p, c, r, d] = sum of 1/cnt over edges (src=c*128+p, type r,
    # dst=core*NPC+d) — aggregation contracts over src (node-major x).
    A = np.zeros((NCORES, 128, NCH, NR, NPC), np.float32)
    np.add.at(A, (dst // NPC, src % 128, src // 128, et, dst % NPC), scale_e)
    adj_bits = _to_fp8_bits(A)  # [NCORES, 128, NCH, NR, NPC] uint8
    wo_r = np.zeros((DH + 1, NH, HID), np.float32)
    for h in range(NH):
        wo_r[1:, h, :] = wo[DH * h:DH * (h + 1), :]
    bo_eff = (np.asarray(bo, np.float64) + np.asarray(bv, np.float64) @ np.asarray(wo, np.float64)).astype(np.float32)
    rel_cat = np.concatenate([rgcn_rel[:, r, :, :] for r in range(NR)], axis=2)  # [NL, HID, NR*HID]
    wq_s = (np.asarray(wq, np.float32) * LOG2E).astype(np.float16)

    shared = {
        "lm_w": np.asarray(lm_w, np.float16),
        "lm_b": np.asarray(lm_b, np.float32),
        "ln_g": np.asarray(ln_g, np.float32),
        "ln_b": np.asarray(ln_b, np.float32),
        "root": np.asarray(rgcn_root, np.float16),
        "rel": np.ascontiguousarray(rel_cat).astype(np.float16),
        "rgb": np.asarray(rgcn_bias, np.float32),
        "wq": wq_s,
        "wk": _to_fp8_bits(np.asarray(wk, np.float32)),
        "wv": _to_fp8_bits(np.asarray(wv, np.float32)),
        "wo": wo_r.astype(np.float16),
        "boe": bo_eff,
        "bn_g": np.asarray(bn_g, np.float32),
        "bn_b": np.asarray(bn_b, np.float32),
        "cls_w": np.asarray(cls_w, np.float16),
        "cls_b": np.asarray(cls_b, np.float32),
        "eye": np.eye(128, dtype=np.float16),
    }

    in_maps = []
    for c in range(NCORES):
        m = dict(shared)
        m["outT"] = np.ascontiguousarray(outT[:, c * NPC:(c + 1) * NPC])
        m["adj"] = np.ascontiguousarray(adj_bits[c])
        in_maps.append(m)

    if "nc" not in _CACHE:
        nc = bacc.Bacc("TRN2")
        nc.num_devices = NCORES
        _CACHE["nc"] = build(nc)
    nc = _CACHE["nc"]

    res = run_bass_kernel_spmd(nc, in_maps, core_ids=list(range(NCORES)))
    LAST_RESULT = res
    y = np.concatenate([res.results[c]["y"] for c in range(NCORES)], axis=0)
    return y.reshape(1, N, NH).astype(np.float32)


# revision 29
# speedup vs baseline: 1.2674x; 1.1289x over previous
"""Trainium2 Bass kernel for nn_CrossModelAttention (gnn_message_passing).

Distribution (8 NeuronCores, one SPMD NEFF):
  - lm head + LayerNorm: node-sharded (256 nodes/core), all local.
  - RGCN x2: message passing: each core transforms only its OWN 256 nodes
    (per-relation GEMM), then computes partial messages to ALL 2048 dst via
    fp8 DoubleRow matmuls against a host-built dense adjacency block
    (mean-normalization folded in); ReduceScatter(sum) returns each core its
    own dst slice. Root term overlaps the RS.
  - Attention: query-sharded. Scores for ALL 8 heads per key chunk with one
    matmul against a zero-blocked Q operand (full 128 contraction). exp is
    split across ACT (native, fp8 out) and DVE/GPSIMD (Schraudolph bit-trick
    into fp8). Numerator via fp8 DoubleRow matmuls (2 key chunks per
    instruction), denominators from an augmented ones-row in V.
  - Residual + BatchNorm: 1KB AllReduce of channel partials, local norm +
    classifier.

Layouts: activations feature-major ("T layout"). Heavy matmuls fp8/fp16,
PSUM/stats fp32.
"""

import os
import sys

if "/opt/trn_rl_repo" not in sys.path:
    sys.path.insert(0, "/opt/trn_rl_repo")

import numpy as np

import concourse.bacc as bacc
import concourse.bass as bass
import concourse.mybir as mybir
import concourse.tile as tile
from concourse.bass_utils import run_bass_kernel_spmd

F32 = mybir.dt.float32
F16 = mybir.dt.float16
F8 = mybir.dt.float8e4
I8 = mybir.dt.int8
U8 = mybir.dt.uint8
AF = mybir.ActivationFunctionType
OP = mybir.AluOpType
DR = mybir.MatmulPerfMode.DoubleRow

N = 2048          # nodes (B*S)
D = 1024          # input dim
HID = 128
NR = 3            # relations
NL = 2            # rgcn layers
NH = 8            # heads
DH = 16
DHA = 20       # augmented V width (ones + 16 dims + pad to 4-byte multiple)
NCORES = 8
NPC = N // NCORES  # nodes per core = 256
NCH = N // 128     # key chunks = 16
EPS = 1e-5
LN2 = 0.6931471805599453
LOG2E = 1.4426950408889634
# Schraudolph constant for 2^x in fp8e4m3 bits: bits = x*8 + SCHB
SCHB = 8.0 * (7.0 - 0.0450) + 0.5

# exp engine assignment per (chunk, group) unit index u = c*2+g  (32 units):
# 'act' = native exp on ACT (fp8 out); 'dve' = Schraudolph 2-pass on DVE.
EXP_ENG = {}
for _u in range(32):
    EXP_ENG[_u] = 'act' if _u % 16 < 9 else 'dve'

LAST_RESULT = None  # BassKernelResults of the most recent run (for test harness)


def _ensure_profile_hook():
    """Install the NTFF profile hook if boot() could not. Only matters when
    BASS_TRACE=1; degrades silently otherwise."""
    try:
        try:
            import antenv.axon_hooks as ah
        except ImportError:
            import types
            import antenv
            ah = types.ModuleType("antenv.axon_hooks")
            _box = [None]
            ah.set_axon_ntff_profile_hook = lambda h: _box.__setitem__(0, h)
            ah.get_axon_ntff_profile_hook = lambda: _box[0]
            sys.modules["antenv.axon_hooks"] = ah
            antenv.axon_hooks = ah
        if ah.get_axon_ntff_profile_hook() is None:
            from trn_agent_boot.trn_boot import _ntff_profile_via_ctypes
            hook = _ntff_profile_via_ctypes("/opt/axon/libaxon_pjrt.so")
            if hook is not None:
                ah.set_axon_ntff_profile_hook(hook)
    except Exception:
        pass


def _bcast_ap(dram_ap, parts, free):
    return bass.AP(tensor=dram_ap.tensor, offset=dram_ap.offset, ap=[[0, parts], [1, free]])


def _gather_cc_ap(cc):
    """AP over cc_out [R, 128, NPC] reading as [128 feat, R*NPC nodes]."""
    return bass.AP(tensor=cc[:].tensor, offset=0,
                   ap=[[NPC, 128], [128 * NPC, NCORES], [1, NPC]])


def build(nc):
    groups = [list(range(NCORES))]

    # ---------------- dram tensors ----------------
    outT_d = nc.dram_tensor("outT", [D, NPC], F16, kind="ExternalInput")
    lmw_d = nc.dram_tensor("lm_w", [D, HID], F16, kind="ExternalInput")
    lmb_d = nc.dram_tensor("lm_b", [HID], F32, kind="ExternalInput")
    lng_d = nc.dram_tensor("ln_g", [HID], F32, kind="ExternalInput")
    lnb_d = nc.dram_tensor("ln_b", [HID], F32, kind="ExternalInput")
    root_d = nc.dram_tensor("root", [NL, HID, HID], F16, kind="ExternalInput")
    rel_d = nc.dram_tensor("rel", [NL, HID, NR * HID], F16, kind="ExternalInput")
    rgb_d = nc.dram_tensor("rgb", [NL, HID], F32, kind="ExternalInput")
    wq_d = nc.dram_tensor("wq", [HID, HID], F16, kind="ExternalInput")
    wk_d = nc.dram_tensor("wk", [HID, HID], U8, kind="ExternalInput")
    wv_d = nc.dram_tensor("wv", [HID, HID], U8, kind="ExternalInput")
    wo_d = nc.dram_tensor("wo", [DH + 1, NH, HID], F16, kind="ExternalInput")
    boe_d = nc.dram_tensor("boe", [HID], F32, kind="ExternalInput")
    bng_d = nc.dram_tensor("bn_g", [HID], F32, kind="ExternalInput")
    bnb_d = nc.dram_tensor("bn_b", [HID], F32, kind="ExternalInput")
    clsw_d = nc.dram_tensor("cls_w", [HID, NH], F16, kind="ExternalInput")
    clsb_d = nc.dram_tensor("cls_b", [NH], F32, kind="ExternalInput")
    adj_d = nc.dram_tensor("adj", [128, NCH, NR, NPC], U8, kind="ExternalInput")
    eye_d = nc.dram_tensor("eye", [128, 128], F16, kind="ExternalInput")

    y_d = nc.dram_tensor("y", [NPC, NH], F32, kind="ExternalOutput")

    # collectives buffers (3 fp8 AllGathers: lm node-major, g1 node-major,
    # g2 feature-major), BN AllReduce, and a warm-up AllReduce.
    agx_in = [nc.dram_tensor(f"agxi{i}", [128, NPC], U8, kind="Internal")
              for i in range(3)]
    agx_out = [nc.dram_tensor(f"agxo{i}", [NCORES, 128, NPC], U8, kind="Internal",
                              addr_space="Shared") for i in range(3)]
    bn_in = nc.dram_tensor("bni", [128, 2], F32, kind="Internal")
    bn_out = nc.dram_tensor("bno", [NCORES, 128, 2], F32, kind="Internal",
                            addr_space="Shared")

    with tile.TileContext(nc) as tc:
        with tc.tile_pool(name="const", bufs=1) as cst, \
             tc.tile_pool(name="persist", bufs=1) as per, \
             tc.tile_pool(name="work", bufs=2) as wk, \
             tc.tile_pool(name="small", bufs=2) as sm:

            # ---------------- constants to SBUF ----------------
            outT_sb = cst.tile([128, D // 128, NPC], F16)
            nc.sync.dma_start(out=outT_sb[:], in_=outT_d[:].rearrange("(k p) n -> p k n", p=128))
            lmw_sb = cst.tile([128, D // 128, HID], F16)
            nc.sync.dma_start(out=lmw_sb[:], in_=lmw_d[:].rearrange("(k p) f -> p k f", p=128))
            adj_sb = cst.tile([128, NCH, NR, NPC], F8)
            nc.sync.dma_start(out=adj_sb[:].bitcast(U8), in_=adj_d[:])
            eye_sb = cst.tile([128, 128], F16)
            nc.sync.dma_start(out=eye_sb[:], in_=eye_d[:])

            root_sb = cst.tile([128, NL, HID], F16)
            nc.gpsimd.dma_start(out=root_sb[:], in_=root_d[:].rearrange("l k f -> k l f"))
            rel_sb = cst.tile([128, NL, NR * HID], F16)
            nc.gpsimd.dma_start(out=rel_sb[:], in_=rel_d[:].rearrange("l k f -> k l f"))
            wq_sb = cst.tile([128, HID], F16)
            nc.gpsimd.dma_start(out=wq_sb[:], in_=wq_d[:])
            wk_sb = cst.tile([128, HID], F8)
            nc.gpsimd.dma_start(out=wk_sb[:].bitcast(U8), in_=wk_d[:])
            wv_sb = cst.tile([128, HID], F8)
            nc.gpsimd.dma_start(out=wv_sb[:].bitcast(U8), in_=wv_d[:])
            wo_sb = cst.tile([DH + 1, NH, HID], F16)
            nc.gpsimd.dma_start(out=wo_sb[:], in_=wo_d[:])
            cls_sb = cst.tile([128, NH], F16)
            nc.gpsimd.dma_start(out=cls_sb[:], in_=clsw_d[:])

            def vec128(d):
                t = cst.tile([128, 1], F32, tag=f"v_{d.name}")
                nc.gpsimd.dma_start(out=t[:], in_=d[:, None])
                return t
            lmb_sb = vec128(lmb_d)
            lng_sb = vec128(lng_d)
            lnb_sb = vec128(lnb_d)
            bng_sb = vec128(bng_d)
            bnb_sb = vec128(bnb_d)
            boe_sb = vec128(boe_d)
            rgb_sb = cst.tile([128, NL], F32)
            nc.gpsimd.dma_start(out=rgb_sb[:], in_=rgb_d[:].rearrange("l f -> f l"))
            clsb_bc = cst.tile([128, NH], F32)
            nc.gpsimd.dma_start(out=clsb_bc[:], in_=_bcast_ap(clsb_d[:], 128, NH))

            ones_col = cst.tile([128, 1], F32)
            nc.vector.memset(ones_col[:], 1.0)
            ones_1x128 = cst.tile([1, 128], F16)
            nc.vector.memset(ones_1x128[:], 1.0)
            ones_1x17 = cst.tile([1, DH + 1], F16)
            nc.vector.memset(ones_1x17[:], 1.0)
            eps1 = cst.tile([1, 1], F32)
            nc.vector.memset(eps1[:], EPS)
            eps128 = cst.tile([128, 1], F32)
            nc.vector.memset(eps128[:], EPS)

            # persistent activation tiles
            lmT_own = per.tile([128, NPC], F32, tag="lmT_own")    # LN output fp32
            lm16_own = per.tile([128, NPC], F16, tag="lm16_own")
            qb16 = per.tile([128, NH, NPC], F16, tag="qb16")      # blocked Q
            x8nm = [per.tile([128, NCH, 128], F8, tag=f"x8nm{l}", name=f"x8nm{l}")
                    for l in range(NL)]   # node-major x (full graph, fp8)
            gT8 = per.tile([128, N], F8, tag="gT8")               # final graph feats (full)
            k16 = per.tile([128, N], F16, tag="k16")              # K all heads [hd, key]
            vaug8 = per.tile([128, NCH, NH, DHA], F8, tag="vaug8")
            e8 = per.tile([128, NCH, NH * NPC], F8, tag="e8")     # [128, 16, 2048]
            num_sb = per.tile([DHA, NH, NPC], F32, tag="num_sb")
            rden = per.tile([1, NH, NPC], F32, tag="rden")
            fused = per.tile([128, NPC], F32, tag="fused")

            # =========== phase 1 + RGCN (psum scope A) ===========
            with tc.tile_pool(name="psPre", bufs=2, space="PSUM") as psP, \
                 tc.tile_pool(name="psAgg", bufs=1, space="PSUM") as psAgg, \
                 tc.tile_pool(name="psG", bufs=2, space="PSUM") as psG:

                def big_ps(name):
                    return psP.tile([128, 512], F32, tag="big", name=name)

                # ---- lm head ----
                lm_ps = big_ps("lm")[:, 0:NPC]
                for k in range(D // 128):
                    nc.tensor.matmul(lm_ps, lhsT=lmw_sb[:, k, :], rhs=outT_sb[:, k, :],
                                     start=(k == 0), stop=(k == D // 128 - 1))
                r_sb = wk.tile([128, NPC], F32, tag="r")
                # relu(x + b) on DVE: (x add b) max 0
                nc.vector.tensor_scalar(out=r_sb[:], in0=lm_ps, scalar1=lmb_sb[:],
                                        scalar2=0.0, op0=OP.add, op1=OP.max)
                # ---- LayerNorm over features (partition dim) via PE stats ----
                sq = wk.tile([128, NPC], F32, tag="sq")
                nc.vector.tensor_mul(sq[:], r_sb[:], r_sb[:])
                st_ps = big_ps("st")[0:1, :]
                nc.tensor.matmul(st_ps[:, 0:NPC], lhsT=ones_col[:], rhs=r_sb[:], start=True, stop=True)
                nc.tensor.matmul(st_ps[:, NPC:], lhsT=ones_col[:], rhs=sq[:], start=True, stop=True)
                mu_r = sm.tile([1, NPC], F32, tag="mu")
                nc.vector.tensor_scalar_mul(mu_r[:], st_ps[:, 0:NPC], 1.0 / HID)
                ex2_r = sm.tile([1, NPC], F32, tag="ex2")
                nc.vector.tensor_scalar_mul(ex2_r[:], st_ps[:, NPC:], 1.0 / HID)
                var_r = sm.tile([1, NPC], F32, tag="var")
                nc.vector.tensor_mul(var_r[:], mu_r[:], mu_r[:])
                nc.vector.tensor_sub(var_r[:], ex2_r[:], var_r[:])
                sd_r = sm.tile([1, NPC], F32, tag="sd")
                nc.scalar.activation(out=sd_r[:], in_=var_r[:], func=AF.Sqrt, bias=eps1[:], scale=1.0)
                pk = sm.tile([1, 2 * NPC], F32, tag="pk")
                nc.vector.reciprocal_approx_fast(out=pk[:, 0:NPC], in_=sd_r[:])
                nc.vector.tensor_mul(pk[:, NPC:], mu_r[:], pk[:, 0:NPC])
                pk16 = sm.tile([1, 2 * NPC], F16, tag="pk16")
                nc.vector.tensor_copy(pk16[:], pk[:])
                bc_ps = big_ps("bc")
                nc.tensor.matmul(bc_ps[:], lhsT=ones_1x128[:], rhs=pk16[:], start=True, stop=True)
                nc.vector.tensor_mul(lmT_own[:], r_sb[:], bc_ps[:, 0:NPC])
                nc.vector.tensor_sub(lmT_own[:], lmT_own[:], bc_ps[:, NPC:])
                nc.vector.tensor_scalar(out=lmT_own[:], in0=lmT_own[:], scalar1=lng_sb[:],
                                        scalar2=lnb_sb[:], op0=OP.mult, op1=OP.add)
                nc.vector.tensor_copy(lm16_own[:], lmT_own[:])

                # ---- Q (all heads, one matmul) + blocked-Q build ----
                q_ps = big_ps("q")[:, 0:NPC]
                nc.tensor.matmul(q_ps, lhsT=wq_sb[:], rhs=lm16_own[:], start=True, stop=True)
                q16 = wk.tile([128, NPC], F16, tag="q16")
                nc.vector.tensor_copy(q16[:], q_ps)
                nc.gpsimd.memset(qb16[:], 0.0)
                for h in range(NH):
                    nc.sync.dma_start(out=qb16[DH * h:DH * (h + 1), h, :],
                                      in_=q16[DH * h:DH * (h + 1), :])

                # ---- RGCN layers (aggregate-first, fp8 node-major AllGathers) ----
                def nm_allgather(x16_src, slot, dst_tile):
                    # transpose own 2 chunks to node-major fp8, AllGather to full
                    xo = wk.tile([128, 2, 128], F8, tag="xnm_own")
                    for i in range(2):
                        tr_ps = psP.tile([128, 128], F16, tag="tr", name=f"tr{slot}{i}")
                        nc.tensor.transpose(tr_ps[:], x16_src[:, 128 * i:128 * (i + 1)],
                                            eye_sb[:])
                        nc.vector.tensor_copy(xo[:, i, :], tr_ps[:])
                    nc.sync.dma_start(out=agx_in[slot][:],
                                      in_=xo[:].rearrange("p i f -> p (i f)").bitcast(U8))
                    nc.gpsimd.collective_compute(
                        kind="AllGather", op=OP.bypass, replica_groups=groups,
                        ins=[agx_in[slot][:]], outs=[agx_out[slot][:]])
                    nc.sync.dma_start(
                        out=dst_tile[:].rearrange("p c f -> p (c f)").bitcast(U8)
                            .rearrange("p (r n) -> p r n", r=NCORES),
                        in_=_gather_cc_ap(agx_out[slot]))

                nm_allgather(lm16_own, 0, x8nm[0])

                x16 = lm16_own
                g16 = None
                for l in range(NL):
                    # aggregate: agg_r[f, dst_own] = sum_src x[src, f] * A_r[src, dst]
                    agg_ps = psAgg.tile([128, NR, NPC], F32, tag="agg", name=f"agg{l}")
                    agg_flat = agg_ps[:].rearrange("p r n -> p (r n)")
                    adj_flat = adj_sb[:].rearrange("p c r n -> p c (r n)")
                    for u in range(NCH // 2):
                        for s0, s1 in ((0, 512), (512, NR * NPC)):
                            nc.tensor.matmul(
                                agg_flat[:, s0:s1],
                                lhsT=x8nm[l][:, 2 * u:2 * u + 2, :],
                                rhs=adj_flat[:, 2 * u:2 * u + 2, s0:s1],
                                start=(u == 0), stop=(u == NCH // 2 - 1),
                                perf_mode=DR)
                    agg16 = wk.tile([128, NR, NPC], F16, tag=f"agg16_{l}")
                    nc.vector.tensor_copy(
                        agg16[:].rearrange("p r n -> p (r n)"),
                        agg_ps[:].rearrange("p r n -> p (r n)"))
                    # g = relu(root^T x + b + sum_r W_r^T agg_r)
                    g_ps = psG.tile([128, NPC], F32, tag="g", name=f"g{l}")
                    nc.tensor.matmul(g_ps[:], lhsT=root_sb[:, l, :], rhs=x16[:],
                                     start=True, stop=False)
                    for r in range(NR):
                        nc.tensor.matmul(g_ps[:], lhsT=rel_sb[:, l, 128 * r:128 * (r + 1)],
                                         rhs=agg16[:, r, :],
                                         start=False, stop=(r == NR - 1))
                    g16 = per.tile([128, NPC], F16, tag=f"g16_{l}")
                    nc.vector.tensor_scalar(out=g16[:], in0=g_ps[:], scalar1=rgb_sb[:, l:l + 1],
                                            scalar2=0.0, op0=OP.add, op1=OP.max)
                    if l < NL - 1:
                        nm_allgather(g16, 1, x8nm[l + 1])
                    x16 = g16

                # ---- AllGather final g (fp8, feature-major) ----
                g8 = wk.tile([128, NPC], F8, tag="g8")
                nc.vector.tensor_copy(g8[:], g16[:])
                nc.sync.dma_start(out=agx_in[2][:], in_=g8[:].bitcast(U8))
                nc.gpsimd.collective_compute(
                    kind="AllGather", op=OP.bypass, replica_groups=groups,
                    ins=[agx_in[2][:]], outs=[agx_out[2][:]])
                nc.sync.dma_start(out=gT8[:].bitcast(U8).rearrange("f (r n) -> f r n", r=NCORES),
                                  in_=_gather_cc_ap(agx_out[2]))

                # ---- K (all heads) and V (key-major, fp8, augmented ones) ----
                for j in range(4):
                    k_ps = big_ps(f"k{j}")
                    nc.tensor.matmul(k_ps[:], lhsT=wk_sb[:], rhs=gT8[:, 512 * j:512 * (j + 1)],
                                     start=True, stop=True)
                    nc.vector.tensor_copy(k16[:, 512 * j:512 * (j + 1)], k_ps[:])
                nc.gpsimd.memset(vaug8[:].bitcast(U8), 0)
                nc.gpsimd.memset(vaug8[:, :, :, 0:1].bitcast(U8), 0x38)  # 1.0 in e4m3
                for c in range(NCH):
                    v_ps = big_ps(f"v{c}")[:, 0:HID]
                    nc.tensor.matmul(v_ps, lhsT=gT8[:, 128 * c:128 * (c + 1)], rhs=wv_sb[:],
                                     start=True, stop=True)
                    nc.vector.tensor_copy(
                        vaug8[:, c, :, 1:DH + 1],
                        v_ps.rearrange("p (h d) -> p h d", h=NH))

            # =========== attention scores/exp/num (psum scope B) ===========
            with tc.tile_pool(name="psS", bufs=2, space="PSUM") as psS, \
                 tc.tile_pool(name="psN", bufs=1, space="PSUM") as psN:

                num_ps = psN.tile([DHA, NH, NPC], F32, tag="num", name="num")
                for c in range(NCH):
                    for g in range(2):
                        s_ps = psS.tile([128, 2, 512], F32, tag="sc", name=f"s{c}{g}")
                        for s2 in range(2):
                            nc.tensor.matmul(
                                s_ps[:, s2, :],
                                lhsT=k16[:, 128 * c:128 * (c + 1)],
                                rhs=qb16[:, 4 * g + 2 * s2:4 * g + 2 * s2 + 2, :],
                                start=True, stop=True)
                        # exp over [128, 1024], engine chosen per unit
                        u = 2 * c + g
                        eng = EXP_ENG[u]
                        e_out = e8[:, c, 1024 * g:1024 * (g + 1)]
                        s_in = s_ps[:].rearrange("p a b -> p (a b)")
                        if eng == 'act':
                            nc.scalar.activation(out=e_out, in_=s_in, func=AF.Exp,
                                                 scale=LN2)
                        else:
                            nc.vector.tensor_scalar(out=e_out.bitcast(I8), in0=s_in,
                                                    scalar1=8.0, scalar2=SCHB,
                                                    op0=OP.mult, op1=OP.add)
                    # numerator DR matmuls for completed chunk pairs
                    if c % 2 == 1:
                        jp = c // 2
                        for h in range(NH):
                            nc.tensor.matmul(
                                num_ps[:, h, :],
                                lhsT=vaug8[:, 2 * jp:2 * jp + 2, h, :],
                                rhs=e8[:, 2 * jp:2 * jp + 2, NPC * h:NPC * (h + 1)],
                                start=(jp == 0), stop=(jp == NCH // 2 - 1),
                                perf_mode=DR)

                # denominators -> reciprocal (fp32); stash numerators in SBUF
                for h in range(NH):
                    nc.vector.reciprocal_approx_fast(out=rden[:, h, :], in_=num_ps[0:1, h, :])
                nc.vector.tensor_copy(num_sb[:].rearrange("d h n -> d (h n)"),
                                      num_ps[:].rearrange("d h n -> d (h n)"))

            # =========== attention tails + BN + classifier (psum scope C) ===========
            with tc.tile_pool(name="psT", bufs=2, space="PSUM") as psT, \
                 tc.tile_pool(name="psA", bufs=1, space="PSUM") as psA:
                attn_ps = psA.tile([128, NPC], F32, tag="attn", name="attn")
                for h in range(NH):
                    rd16 = sm.tile([1, NPC], F16, tag="rd16")
                    nc.vector.tensor_copy(rd16[:], rden[:, h, :])
                    dbc_ps = psT.tile([DH + 1, NPC], F32, tag="dbc", name=f"dbc{h}")
                    nc.tensor.matmul(dbc_ps[:], lhsT=ones_1x17[:], rhs=rd16[:],
                                     start=True, stop=True)
                    ctx16 = sm.tile([DH + 1, NPC], F16, tag="ctx16")
                    nc.vector.tensor_mul(ctx16[:], num_sb[0:DH + 1, h, :], dbc_ps[:])
                    nc.tensor.matmul(attn_ps[:], lhsT=wo_sb[:, h, :], rhs=ctx16[:],
                                     start=(h == 0), stop=(h == NH - 1))

                # residual + BatchNorm over nodes (channel stats)
                nc.vector.scalar_tensor_tensor(out=fused[:], in0=attn_ps[:], scalar=boe_sb[:],
                                               in1=lmT_own[:], op0=OP.add, op1=OP.add)
                fsq = wk.tile([128, NPC], F32, tag="fsq")
                nc.vector.tensor_mul(fsq[:], fused[:], fused[:])
                bnp = sm.tile([128, 2], F32, tag="bnp")
                nc.vector.tensor_reduce(bnp[:, 0:1], fused[:], mybir.AxisListType.X, OP.add)
                nc.vector.tensor_reduce(bnp[:, 1:2], fsq[:], mybir.AxisListType.X, OP.add)
                nc.sync.dma_start(out=bn_in[:], in_=bnp[:])
                nc.gpsimd.collective_compute(
                    kind="AllGather", op=OP.bypass, replica_groups=groups,
                    ins=[bn_in[:]], outs=[bn_out[:]])
                bng8 = sm.tile([128, 2, NCORES], F32, tag="bng8")
                nc.sync.dma_start(
                    out=bng8[:],
                    in_=bass.AP(tensor=bn_out[:].tensor, offset=0,
                                ap=[[2, 128], [1, 2], [256, NCORES]]))
                bnst = sm.tile([128, 2], F32, tag="bnst")
                nc.vector.tensor_reduce(bnst[:, 0:1], bng8[:, 0, :], mybir.AxisListType.X, OP.add)
                nc.vector.tensor_reduce(bnst[:, 1:2], bng8[:, 1, :], mybir.AxisListType.X, OP.add)
                mu_c = sm.tile([128, 1], F32, tag="muc")
                nc.vector.tensor_scalar_mul(mu_c[:], bnst[:, 0:1], 1.0 / N)
                var_c = sm.tile([128, 1], F32, tag="varc")
                nc.vector.tensor_scalar_mul(var_c[:], bnst[:, 1:2], 1.0 / N)
                mu2_c = sm.tile([128, 1], F32, tag="mu2c")
                nc.vector.tensor_mul(mu2_c[:], mu_c[:], mu_c[:])
                nc.vector.tensor_sub(var_c[:], var_c[:], mu2_c[:])
                sd_c = sm.tile([128, 1], F32, tag="sdc")
                nc.scalar.activation(out=sd_c[:], in_=var_c[:], func=AF.Sqrt, bias=eps128[:], scale=1.0)
                scl_c = sm.tile([128, 1], F32, tag="sclc")
                nc.vector.reciprocal_approx_fast(out=scl_c[:], in_=sd_c[:])
                nc.vector.tensor_mul(scl_c[:], scl_c[:], bng_sb[:])
                shf_c = sm.tile([128, 1], F32, tag="shfc")
                nc.vector.tensor_mul(shf_c[:], mu_c[:], scl_c[:])
                nc.vector.tensor_sub(shf_c[:], bnb_sb[:], shf_c[:])
                fn16 = wk.tile([128, NPC], F16, tag="fn16")
                nc.vector.tensor_scalar(out=fn16[:], in0=fused[:], scalar1=scl_c[:],
                                        scalar2=shf_c[:], op0=OP.mult, op1=OP.add)
                yv = y_d[:].rearrange("(c p) f -> c p f", p=128)
                for c in range(NPC // 128):
                    lg_ps = psT.tile([128, NH], F32, tag="lg", name=f"lg{c}")
                    nc.tensor.matmul(lg_ps[:], lhsT=fn16[:, 128 * c:128 * (c + 1)], rhs=cls_sb[:],
                                     start=True, stop=True)
                    out_sb = wk.tile([128, NH], F32, tag="outsb")
                    nc.vector.tensor_add(out_sb[:], lg_ps[:], clsb_bc[:])
                    nc.sync.dma_start(out=yv[c], in_=out_sb[:])

    nc.finalize()
    return nc


def _to_fp8_bits(x):
    """float array -> fp8 e4m3 bits as uint8."""
    try:
        import ml_dtypes
        return np.asarray(x).astype(ml_dtypes.float8_e4m3fn).view(np.uint8)
    except Exception:
        x = np.asarray(x, np.float32)
        sign = (x < 0).astype(np.uint8) << 7
        ax = np.abs(x)
        ax = np.clip(ax, 0, 448.0)
        e = np.floor(np.log2(np.maximum(ax, 1e-45))).astype(np.int32)
        e = np.clip(e, -9, 8)
        # normal range e >= -6
        en = np.clip(e, -6, 8)
        m = np.round(ax / (2.0 ** en) * 8).astype(np.int32)
        m = np.where(ax == 0, 0, m)
        # m in [8, 16] for normals; handle rounding overflow
        carry = m >= 16
        en = en + carry
        m = np.where(carry, 8, m)
        sub = en <= -7
        bits = np.where(
            sub,
            np.round(ax / 2.0 ** -9).astype(np.int32),
            ((en + 7) << 3) | (m - 8),
        )
        bits = np.clip(bits, 0, 0x7E)
        return (sign | bits.astype(np.uint8)).astype(np.uint8)


_CACHE = {}


def kernel(output, edge_index, edge_type, lm_w, lm_b, ln_g, ln_b,
           rgcn_root, rgcn_rel, rgcn_bias, wq, bq, wk, bk, wv, bv,
           wo, bo, bn_g, bn_b, cls_w, cls_b):
    global LAST_RESULT
    _ensure_profile_hook()

    output = np.asarray(output, np.float32)
    src = np.asarray(edge_index[0]).astype(np.int64)
    dst = np.asarray(edge_index[1]).astype(np.int64)
    et = np.asarray(edge_type).astype(np.int64)
    bq = np.asarray(bq, np.float32)
    bk = np.asarray(bk, np.float32)
    if np.abs(bq).max() > 0 or np.abs(bk).max() > 0:
        raise NotImplementedError("nonzero bq/bk not supported by this kernel")

    # ---- host-side layout prep (index math only) ----
    outT = np.ascontiguousarray(output.reshape(N, D).T).astype(np.float16)  # [D, N]
    cnt = np.zeros((N, NR), np.float32)
    np.add.at(cnt, (dst, et), 1.0)
    scale_e = (1.0 / np.maximum(cnt, 1.0))[dst, et].astype(np.float32)
    # adjacency per core: [core, p, r, i, dst]: src = core*256 + i*128 + p
    # adj[core, p, c, r, d] = sum of 1/cnt over edges (src=c*128+p, type r,
    # dst=core*NPC+d) — aggregation contracts over src (node-major x).
    A = np.zeros((NCORES, 128, NCH, NR, NPC), np.float32)
    np.add.at(A, (dst // NPC, src % 128, src // 128, et, dst % NPC), scale_e)
    adj_bits = _to_fp8_bits(A)  # [NCORES, 128, NCH, NR, NPC] uint8
    wo_r = np.zeros((DH + 1, NH, HID), np.float32)
    for h in range(NH):
        wo_r[1:, h, :] = wo[DH * h:DH * (h + 1), :]
    bo_eff = (np.asarray(bo, np.float64) + np.asarray(bv, np.float64) @ np.asarray(wo, np.float64)).astype(np.float32)
    rel_cat = np.concatenate([rgcn_rel[:, r, :, :] for r in range(NR)], axis=2)  # [NL, HID, NR*HID]
    wq_s = (np.asarray(wq, np.float32) * LOG2E).astype(np.float16)

    shared = {
        "lm_w": np.asarray(lm_w, np.float16),
        "lm_b": np.asarray(lm_b, np.float32),
        "ln_g": np.asarray(ln_g, np.float32),
        "ln_b": np.asarray(ln_b, np.float32),
        "root": np.asarray(rgcn_root, np.float16),
        "rel": np.ascontiguousarray(rel_cat).astype(np.float16),
        "rgb": np.asarray(rgcn_bias, np.float32),
        "wq": wq_s,
        "wk": _to_fp8_bits(np.asarray(wk, np.float32)),
        "wv": _to_fp8_bits(np.asarray(wv, np.float32)),
        "wo": wo_r.astype(np.float16),
        "boe": bo_eff,
        "bn_g": np.asarray(bn_g, np.float32),
        "bn_b": np.asarray(bn_b, np.float32),
        "cls_w": np.asarray(cls_w, np.float16),
        "cls_b": np.asarray(cls_b, np.float32),
        "eye": np.eye(128, dtype=np.float16),
    }

    in_maps = []
    for c in range(NCORES):
        m = dict(shared)
        m["outT"] = np.ascontiguousarray(outT[:, c * NPC:(c + 1) * NPC])
        m["adj"] = np.ascontiguousarray(adj_bits[c])
        in_maps.append(m)

    if "nc" not in _CACHE:
        nc = bacc.Bacc("TRN2")
        nc.num_devices = NCORES
        _CACHE["nc"] = build(nc)
    nc = _CACHE["nc"]

    res = run_bass_kernel_spmd(nc, in_maps, core_ids=list(range(NCORES)))
    LAST_RESULT = res
    y = np.concatenate([res.results[c]["y"] for c in range(NCORES)], axis=0)
    return y.reshape(1, N, NH).astype(np.float32)
